# revision 43
# baseline (speedup 1.0000x reference)
"""Trainium2 Bass kernel for nn_DenseEdgeEncoder.

Computes, for B=16 graphs of N=256 nodes with 4096 edges each:
    out[b, i, j, :] = edge_attr[e]      if edge e = (i, j) in graph b
                      emb_table[1]      if i == j (self-loop fill)
                      emb_table[2]      otherwise
(the reference's scatter + embedding-lookup formulation reduces to this;
duplicate edges would scatter-add, which the delta-add below preserves).

Strategy (data-parallel over B, 2 graphs per core on 8 cores):
  1. host: route each graph's edges to its core; convert (src, dst) to flat
     row ids of the dense [N*N, D] per-graph image; fold the diagonal fill in
     as N extra tokens; pre-subtract the background vector v2 = emb_table[2]
     from every token payload (scatter is an ADD on top of the background).
  2. device, per core (out = 2 graphs = 32 MiB):
     - sync (HWDGE): broadcast v2, then write the v2 background CHUNK-major
       (chunk c = scatter view c's row span), chunk 0 in widen-tier pieces.
     - vector (DVE): widen the v2 row across one chunk's span in tiers.
     - gpsimd (SWDGE): load token payloads + int16 indices; PREPARE all
       dma_scatter_add descriptor sets up front (Q7 descriptor generation
       runs under the background writes; one SWDGE queue per view mod 4),
       then TRIGGER view v's scatter the moment chunk v's background lands.
       Only the last chunk's scatter drain is serialized after the writes.
  3. host: stack per-core outputs to [16, 256, 256, 64].
"""

import os
from contextlib import ExitStack

import numpy as np

import concourse.bacc as bacc
from concourse import mybir
from concourse.bass_utils import run_bass_kernel_spmd
from concourse.bass_interp import get_hw_module
from concourse._compat import cdiv

B = 16
N = 256
D = 64
NCORES = 8
GPC = B // NCORES  # graphs per core = 2
NROWS_G = N * N  # 65536 rows per graph
NQ = 4  # SWDGE queues (ucode max); view v preps/triggers on queue v % NQ

# defaults for the graded path
QSPLIT = 2  # scatter views per graph (view span = NROWS_G/QSPLIT rows)
SORT_ROWS = False  # random row order measured faster than sorted on HW

# background widen tiers, cumulative ends (exclusive) in D-blocks of the
# chunk span: lets chunk-0 writes start while later tiers still widen
TIERS_BY_QSPLIT = {
    1: (16, 64, 160, 320, 512),
    2: (16, 64, 160, 256),
    4: (16, 48, 128),
    8: (16, 64),
}

LAST_EXEC_NS = None
LAST_RESULTS = None


def _pack_bucket(rows, deltas, cap, vspan):
    """Pack one (graph-slot, view) bucket for dma_scatter_add.

    rows: int array in [0, vspan) - target rows in the view.
    deltas: [len(rows), D] f32 payload minus background.
    cap: uniform token capacity (multiple of 128).

    Returns (src [128, (cap/128)*D] f32, idx [128, cap/16] int16).
    Padding tokens add 0.0 to a row unused by real tokens (safe under the
    SDMA read-modify-write with no same-row concurrency).
    """
    n = len(rows)
    assert n <= cap
    cols = cap // 128
    cols16 = cap // 16

    # find an unused row for the zero-delta padding tokens
    used = np.zeros(vspan, bool)
    used[rows] = True
    pad_row = int(np.argmin(used))  # first unused row (vspan >> n always)

    rows_p = np.full(cap, pad_row, np.int64)
    rows_p[:n] = rows
    deltas_p = np.zeros((cap, D), np.float32)
    deltas_p[:n] = deltas

    # src: token i lives at [i % 128, (i // 128)*D : ...]
    src = np.ascontiguousarray(
        deltas_p.reshape(cols, 128, D).transpose(1, 0, 2)
    ).reshape(128, cols * D)
    # idx: token i at [i % 16, i // 16], replicated to all 8 gpsimd groups
    idx2 = rows_p.astype(np.int16).reshape(cols16, 16)
    idx = np.ascontiguousarray(idx2.T)
    idx_rep = np.ascontiguousarray(np.tile(idx, (8, 1)))
    return src, idx_rep


def _route(edge_attr, emb_table, edge_index, batch_vec, qsplit=QSPLIT,
           sort=SORT_ROWS):
    """Host-side routing: per-core scatter buckets keyed (core, slot, q)."""
    src, dst = np.asarray(edge_index[0]), np.asarray(edge_index[1])
    batch_vec = np.asarray(batch_vec)
    edge_attr = np.asarray(edge_attr, dtype=np.float32)
    emb_table = np.asarray(emb_table, dtype=np.float32)
    vspan = NROWS_G // qsplit

    counts = np.bincount(batch_vec, minlength=B)
    starts = np.cumsum(counts) - counts
    g = batch_vec[src]
    ls = src - starts[g]
    ld = dst - starts[g]
    ok = (ls >= 0) & (ls < N) & (ld >= 0) & (ld < N)  # jax drops OOB scatters
    g, ls, ld = g[ok], ls[ok], ld[ok]
    ea = edge_attr[ok]

    v1, v2 = emb_table[1], emb_table[2]
    delta_e = ea - v2[None, :]
    delta_d = (v1 - v2)[None, :].repeat(N, axis=0)
    diag_rows = np.arange(N) * (N + 1)

    buckets = {}
    row = ls * N + ld
    for core in range(NCORES):
        for slot in range(GPC):
            gb = core * GPC + slot
            m = g == gb
            r_all = np.concatenate([row[m], diag_rows])
            d_all = np.concatenate([delta_e[m], delta_d], axis=0)
            if sort:
                # ascending rows -> the scatter drain walks HBM in address
                # order (row-buffer locality)
                order = np.argsort(r_all, kind="stable")
                r_all, d_all = r_all[order], d_all[order]
            for q in range(qsplit):
                hm = (r_all >= q * vspan) & (r_all < (q + 1) * vspan)
                buckets[(core, slot, q)] = (r_all[hm] - q * vspan, d_all[hm])

    cap = max(len(r) for r, _ in buckets.values())
    cap = cdiv(max(cap, 128), 128) * 128
    return buckets, cap, emb_table


def build_in_maps(buckets, cap, emb_np, qsplit=QSPLIT):
    vspan = NROWS_G // qsplit
    in_maps = []
    for core in range(NCORES):
        srcs, idxs = [], []
        for slot in range(GPC):
            for q in range(qsplit):
                src, idxr = _pack_bucket(*buckets[(core, slot, q)], cap, vspan)
                srcs.append(src)
                idxs.append(idxr)
        in_maps.append(
            {
                "emb": emb_np,
                "src": np.ascontiguousarray(np.concatenate(srcs, axis=1)),
                "idx": np.ascontiguousarray(np.concatenate(idxs, axis=1)),
            }
        )
    return in_maps


def _build_program(
    cap,
    reps=1,
    qsplit=QSPLIT,
    do_bg=True,
    do_scat=True,
    internal_out=False,
    use_trigger=True,
    bg_order="chunk",
):
    """Build the per-core program. reps>1 repeats the whole body (used only
    for benchmarking: rep r's background waits for rep r-1's scatters).
    do_bg/do_scat/internal_out are bench-only knobs: drop the background
    writes or the scatters, and keep the 32 MiB out tensor device-local
    (Internal) with a tiny dummy ExternalOutput so per-run host I/O is small.

    Semaphore discipline: waits only ever target a semaphore's FULL value at
    that point (per-DMA completions interleave across the 16 SDMA engines, so
    intermediate values can be mixtures of several DMAs).
    """
    vspan = NROWS_G // qsplit
    nview = GPC * qsplit
    tiers = TIERS_BY_QSPLIT[qsplit]
    cols = cap // 128
    cols16 = cap // 16
    bounds = [0] + [t * D for t in tiers]  # tier boundaries in elems

    nc = bacc.Bacc(
        "TRN2",
        target_bir_lowering=False,
        debug=False,
        num_devices=NCORES,
        # all prepared scatters must fit in the SWDGE rings at once
        # (nview/NQ pending per queue)
        dynamic_dma_scratch_size=65536,
        num_swdge_queues=NQ,
    )
    emb_t = nc.dram_tensor("emb", [3, D], mybir.dt.float32, kind="ExternalInput").ap()
    src_t = nc.dram_tensor(
        "src", [128, nview * cols * D], mybir.dt.float32, kind="ExternalInput"
    ).ap()
    idx_t = nc.dram_tensor(
        "idx", [128, nview * cols16], mybir.dt.int16, kind="ExternalInput"
    ).ap()
    out_kind = "Internal" if internal_out else "ExternalOutput"
    out_t = nc.dram_tensor(
        "out", [GPC * NROWS_G, D], mybir.dt.float32, kind=out_kind
    ).ap()
    dum_t = (
        nc.dram_tensor("dum", [1, 1], mybir.dt.float32, kind="ExternalOutput").ap()
        if internal_out
        else None
    )
    if internal_out:
        # unused input, shape-compatible with dum: lets the bench runner
        # chain k executions (dum_i -> chain_{i+1}) inside one jit call
        nc.dram_tensor("chain", [1, 1], mybir.dt.float32, kind="ExternalInput")
    # contiguous chunks: chunk c = out rows [c*vspan, (c+1)*vspan) == exactly
    # scatter view c; partition p holds vspan/128 consecutive rows
    out_chunks = out_t.rearrange(
        "(c p w) d -> c p (w d)", c=nview, p=128, w=vspan // 128
    )
    bg_width = (vspan // 128) * D  # one chunk's span per partition, in f32

    nc.reset()

    with (
        ExitStack() as stack,
        nc.sbuf_tensor([128, bg_width], mybir.dt.float32) as bg,
        nc.sbuf_tensor([128, nview * cols * D], mybir.dt.float32) as pay,
        nc.sbuf_tensor([128, nview * cols16], mybir.dt.int16) as idx,
        nc.semaphore() as s_load,
        nc.semaphore() as s_pay,
        nc.semaphore() as s_scat,
        nc.semaphore() as s_prep,
        nc.semaphore() as s_dum,
        nc.Block() as block,
    ):
        s_tier = [
            stack.enter_context(nc.semaphore(name=f"s_t{i}"))
            for i in range(len(tiers))
        ]
        s_bgc = [
            stack.enter_context(nc.semaphore(name=f"s_bgc{i}")) for i in range(nview)
        ]

        # chunk 0 is written in widen-tier pieces (16 incs each);
        # later chunks are single whole-chunk DMAs (16 incs each)
        if bg_order == "chunk":
            bgc_full = [16 * len(tiers)] + [16] * (nview - 1)
        else:
            bgc_full = [16 * len(tiers)] * nview

        def _wait_prev_rep(q, r):
            if r == 0:
                return
            if do_scat:
                q.wait_ge(s_scat, 16 * nview * r)
            else:
                for c in range(nview):
                    q.wait_ge(s_bgc[c], bgc_full[c] * r)

        @block.sync
        def _(sync):
            for r in range(reps):
                if do_bg:
                    # benchmark mode: previous rep's scatters must finish
                    # before overwriting their rows (and the bg/pay tiles)
                    _wait_prev_rep(sync, r)
                    # v2 row broadcast into all 128 partitions
                    sync.dma_start(
                        out=bg[:, 0:D], in_=emb_t[2:3, :].to_broadcast([128, D])
                    ).then_inc(s_load, 16)
                    if bg_order == "chunk":
                        # chunk 0: tier pieces start while tiers still widen
                        for t in range(len(tiers)):
                            lo, hi = bounds[t], bounds[t + 1]
                            sync.wait_ge(s_tier[t], r + 1)
                            sync.dma_start(
                                out=out_chunks[0][:, lo:hi], in_=bg[:, lo:hi]
                            ).then_inc(s_bgc[0], 16)
                        # later chunks: whole-chunk writes (widen fully done)
                        for c in range(1, nview):
                            sync.dma_start(
                                out=out_chunks[c][:, :], in_=bg[:, :]
                            ).then_inc(s_bgc[c], 16)
                    else:  # tier-major (original): all chunks per tier
                        for t in range(len(tiers)):
                            lo, hi = bounds[t], bounds[t + 1]
                            sync.wait_ge(s_tier[t], r + 1)
                            for c in range(nview):
                                sync.dma_start(
                                    out=out_chunks[c][:, lo:hi], in_=bg[:, lo:hi]
                                ).then_inc(s_bgc[c], 16)
            if internal_out:
                if do_scat:
                    sync.wait_ge(s_scat, 16 * nview * reps)
                elif do_bg:
                    for c in range(nview):
                        sync.wait_ge(s_bgc[c], bgc_full[c] * reps)
                src_dum = bg if do_bg else pay
                sync.dma_start(out=dum_t[:, :], in_=src_dum[0:1, 0:1]).then_inc(
                    s_dum, 16
                )

        if do_bg:

            @block.vector
            def _(vector):
                for r in range(reps):
                    vector.wait_ge(s_load, 16 * (r + 1))
                    v2blk = bg[:, 0:D].rearrange("p (x d) -> p x d", x=1)
                    prev = 1  # first tier's copy starts after the v2 block
                    for t in range(len(tiers)):
                        lo, hi = prev * D, bounds[t + 1]
                        vector.tensor_copy(
                            out=bg[:, lo:hi].rearrange("p (x d) -> p x d", d=D),
                            in_=v2blk.to_broadcast([128, tiers[t] - prev, D]),
                        ).then_inc(s_tier[t], 1)
                        prev = tiers[t]

        if do_scat:

            @block.gpsimd
            def _(gpsimd):
                for r in range(reps):
                    if r > 0:
                        # pay/idx tiles are read by the prev rep's scatters
                        gpsimd.wait_ge(s_scat, 16 * nview * r)
                    gpsimd.dma_start(out=pay[:], in_=src_t[:, :]).then_inc(s_pay, 16)
                    gpsimd.dma_start(out=idx[:], in_=idx_t[:, :]).then_inc(s_pay, 16)
                    gpsimd.wait_ge(s_pay, 32 * (r + 1))
                    if use_trigger:
                        # prepare all scatters now: Q7 descriptor generation
                        # runs while sync is still writing the background
                        for v in range(nview):
                            gpsimd.dma_scatter_add(
                                out_ap=out_t[v * vspan : (v + 1) * vspan, :],
                                in_ap=pay[
                                    :, v * cols * D : (v + 1) * cols * D
                                ].rearrange("p (c d) -> p c d", d=D),
                                idxs_ap=idx[:, v * cols16 : (v + 1) * cols16],
                                num_idxs=cap,
                                num_idxs_reg=cap,
                                elem_size=D,
                                prepare_only=True,
                                sem=s_scat,
                                queue_num=v % NQ,
                            ).then_inc(s_prep, 1)
                        gpsimd.wait_ge(s_prep, nview * (r + 1))
                        # fire scatter v as soon as chunk v's bg landed
                        # (per-queue FIFO: queue q holds preps q, q+NQ, ...,
                        # and is triggered in exactly that order)
                        for v in range(nview):
                            if do_bg:
                                gpsimd.wait_ge(s_bgc[v], bgc_full[v] * (r + 1))
                            gpsimd.trigger_dma(count=1, queue_num=v % NQ)
                    else:
                        for v in range(nview):
                            if do_bg:
                                gpsimd.wait_ge(s_bgc[v], bgc_full[v] * (r + 1))
                            gpsimd.dma_scatter_add(
                                out_ap=out_t[v * vspan : (v + 1) * vspan, :],
                                in_ap=pay[
                                    :, v * cols * D : (v + 1) * cols * D
                                ].rearrange("p (c d) -> p c d", d=D),
                                idxs_ap=idx[:, v * cols16 : (v + 1) * cols16],
                                num_idxs=cap,
                                num_idxs_reg=cap,
                                elem_size=D,
                            ).then_inc(s_scat, 16)
                # triggered drains must land before the program quiesces
                gpsimd.wait_ge(s_scat, 16 * nview * reps)

    nc.compile()
    nc.m = get_hw_module(nc.m)
    return nc


def _pack_bucket_compose(rows, deltas, cap, vspan):
    """Pack one chunk bucket for the SBUF-destination scatter.

    idx encoding for sbuf_tokens_per_rank=128 (dhi=1): dest partition =
    idx % 128, free-dim slot = idx >> 8, parity bit 7 always 0 (all tokens
    route to out_ap).  Chunk-local row r -> partition r//128, slot r%128:
        idx = (r % 128) << 8 | (r // 128)            (max 32639, int16 ok)
    Padding tokens add 0.0 to a row unused by real tokens (a concurrent
    CCE read-modify-write of a REAL token's row could lose that token's
    add, so zero-payload is not by itself safe).
    """
    n = len(rows)
    assert n <= cap and vspan == 16384
    cols = cap // 128
    cols16 = cap // 16

    used = np.zeros(vspan, bool)
    used[rows] = True
    pad_row = int(np.argmin(used))  # first unused row (vspan >> n always)

    enc = (rows % 128) << 8 | (rows // 128)
    pad_enc = (pad_row % 128) << 8 | (pad_row // 128)
    enc_p = np.full(cap, pad_enc, np.int64)
    enc_p[:n] = enc
    deltas_p = np.zeros((cap, D), np.float32)
    deltas_p[:n] = deltas

    src = np.ascontiguousarray(
        deltas_p.reshape(cols, 128, D).transpose(1, 0, 2)
    ).reshape(128, cols * D)
    idx2 = enc_p.astype(np.int16).reshape(cols16, 16)
    idx = np.ascontiguousarray(idx2.T)
    idx_rep = np.ascontiguousarray(np.tile(idx, (8, 1)))
    return src, idx_rep


def build_in_maps_compose(buckets, cap, emb_np):
    in_maps = []
    for core in range(NCORES):
        srcs, idxs = [], []
        for slot in range(GPC):
            for q in range(4):
                src, idxr = _pack_bucket_compose(
                    *buckets[(core, slot, q)], cap, NROWS_G // 4
                )
                srcs.append(src)
                idxs.append(idxr)
        in_maps.append(
            {
                "emb": emb_np,
                "src": np.ascontiguousarray(np.concatenate(srcs, axis=1)),
                "idx": np.ascontiguousarray(np.concatenate(idxs, axis=1)),
            }
        )
    return in_maps


def _build_program_compose(cap, reps=1, internal_out=False):
    """SBUF-compose pipeline: never touches HBM randomly.

    Per chunk k (8 chunks of 16384 rows per rep; tile t = k % 2):
      vector (DVE): widen v2 into tile t              (after write k-2 done)
      gpsimd (SWDGE): dma_scatter_add token deltas INTO the SBUF tile
        (sbuf_tokens_per_rank=128 parity mode, parity bit 0, out_ap_other
        aliased to out_ap)                            (after widen k)
      sync (HWDGE): one dense 4 MiB write tile -> out chunk k
    HBM sees only the sequential chunk writes + the one-time payload loads;
    all token placement happens SBUF->SBUF through the SDMA CCE adders.
    """
    qsplit = 4
    vspan = NROWS_G // qsplit  # 16384
    nview = GPC * qsplit  # 8 chunks per core
    slots = vspan // 128  # 128 rows per partition per chunk
    cols = cap // 128
    cols16 = cap // 16

    nc = bacc.Bacc(
        "TRN2",
        target_bir_lowering=False,
        debug=False,
        num_devices=NCORES,
        dynamic_dma_scratch_size=65536,
    )
    emb_t = nc.dram_tensor("emb", [3, D], mybir.dt.float32, kind="ExternalInput").ap()
    src_t = nc.dram_tensor(
        "src", [128, nview * cols * D], mybir.dt.float32, kind="ExternalInput"
    ).ap()
    idx_t = nc.dram_tensor(
        "idx", [128, nview * cols16], mybir.dt.int16, kind="ExternalInput"
    ).ap()
    out_kind = "Internal" if internal_out else "ExternalOutput"
    out_t = nc.dram_tensor(
        "out", [GPC * NROWS_G, D], mybir.dt.float32, kind=out_kind
    ).ap()
    dum_t = (
        nc.dram_tensor("dum", [1, 1], mybir.dt.float32, kind="ExternalOutput").ap()
        if internal_out
        else None
    )
    if internal_out:
        nc.dram_tensor("chain", [1, 1], mybir.dt.float32, kind="ExternalInput")
    out_chunks = out_t.rearrange("(c p w) d -> c p (w d)", c=nview, p=128, w=slots)

    nc.reset()

    K = nview * reps  # global chunk counter

    with (
        nc.sbuf_tensor([128, slots * D], mybir.dt.float32) as tile0,
        nc.sbuf_tensor([128, slots * D], mybir.dt.float32) as tile1,
        nc.sbuf_tensor([128, D], mybir.dt.float32) as v2t,
        nc.sbuf_tensor([128, nview * cols * D], mybir.dt.float32) as pay,
        nc.sbuf_tensor([128, nview * cols16], mybir.dt.int16) as idx,
        nc.semaphore() as s_ld,
        nc.semaphore() as s_w0,
        nc.semaphore() as s_w1,
        nc.semaphore() as s_sc0,
        nc.semaphore() as s_sc1,
        nc.semaphore() as s_wr0,
        nc.semaphore() as s_wr1,
        nc.semaphore() as s_dum,
        nc.Block() as block,
    ):
        tiles = [tile0, tile1]
        s_wide = [s_w0, s_w1]
        s_sc = [s_sc0, s_sc1]
        s_wr = [s_wr0, s_wr1]

        @block.gpsimd
        def _(gpsimd):
            gpsimd.dma_start(
                out=v2t[:, :], in_=emb_t[2:3, :].to_broadcast([128, D])
            ).then_inc(s_ld, 16)
            gpsimd.dma_start(out=pay[:], in_=src_t[:, :]).then_inc(s_ld, 16)
            gpsimd.dma_start(out=idx[:], in_=idx_t[:, :]).then_inc(s_ld, 16)
            gpsimd.wait_ge(s_ld, 48)
            for k in range(K):
                t, c = k % 2, k % nview
                gpsimd.wait_ge(s_wide[t], k // 2 + 1)
                gpsimd.dma_scatter_add(
                    out_ap=tiles[t][:, :],
                    in_ap=pay[:, c * cols * D : (c + 1) * cols * D].rearrange(
                        "p (c d) -> p c d", d=D
                    ),
                    idxs_ap=idx[:, c * cols16 : (c + 1) * cols16],
                    num_idxs=cap,
                    num_idxs_reg=cap,
                    elem_size=D,
                    sbuf_tokens_per_rank=128,
                    parity_reg=0,
                    out_ap_other=tiles[t][:, :],
                ).then_inc(s_sc[t], 16)

        @block.vector
        def _(vector):
            vector.wait_ge(s_ld, 16)  # v2 loaded (first load on gpsimd queue)
            v2blk = v2t[:, :].rearrange("p (x d) -> p x d", x=1)
            for k in range(K):
                t = k % 2
                if k >= 2:
                    vector.wait_ge(s_wr[t], 16 * (k // 2))
                vector.tensor_copy(
                    out=tiles[t][:, :].rearrange("p (x d) -> p x d", d=D),
                    in_=v2blk.to_broadcast([128, slots, D]),
                ).then_inc(s_wide[t], 1)

        @block.sync
        def _(sync):
            for k in range(K):
                t, c = k % 2, k % nview
                sync.wait_ge(s_sc[t], 16 * (k // 2 + 1))
                sync.dma_start(out=out_chunks[c][:, :], in_=tiles[t][:, :]).then_inc(
                    s_wr[t], 16
                )
            if internal_out:
                sync.wait_ge(s_wr[0], 16 * (K // 2))
                sync.wait_ge(s_wr[1], 16 * (K // 2))
                sync.dma_start(out=dum_t[:, :], in_=v2t[0:1, 0:1]).then_inc(s_dum, 16)

    nc.compile()
    nc.m = get_hw_module(nc.m)
    return nc


def prepare(edge_attr, emb_table, edge_index, batch_vec):
    """Host routing + program build (SBUF-compose path). Returns (nc, in_maps)."""
    buckets, cap, emb_np = _route(
        edge_attr, emb_table, edge_index, batch_vec, qsplit=4, sort=False
    )
    nc = _build_program_compose(cap)
    return nc, build_in_maps_compose(buckets, cap, emb_np)


def kernel(edge_attr, emb_table, edge_index, batch_vec):
    global LAST_EXEC_NS, LAST_RESULTS
    nc, in_maps = prepare(edge_attr, emb_table, edge_index, batch_vec)

    trace = bool(int(os.environ.get("BASSK_TRACE", "0")))
    res = run_bass_kernel_spmd(nc, in_maps, list(range(NCORES)), trace=trace)
    LAST_EXEC_NS = res.exec_time_ns
    LAST_RESULTS = res

    out = np.empty((B, N, N, D), np.float32)
    for core in range(NCORES):
        blockv = res.results[core]["out"].reshape(GPC, N, N, D)
        out[core * GPC : (core + 1) * GPC] = blockv
    return out


# revision 48
# speedup vs baseline: 1.0369x; 1.0369x over previous
"""Trainium2 Bass kernel for nn_DenseEdgeEncoder.

Computes, for B=16 graphs of N=256 nodes with 4096 edges each:
    out[b, i, j, :] = edge_attr[e]      if edge e = (i, j) in graph b
                      emb_table[1]      if i == j (self-loop fill)
                      emb_table[2]      otherwise
(the reference's scatter + embedding-lookup formulation reduces to this;
duplicate edges would scatter-add, which the delta-add below preserves).

Strategy (data-parallel over B, 2 graphs per core on 8 cores), the
"SBUF-compose" pipeline — HBM is only ever touched sequentially:
  1. host: route each graph's edges to its core; convert (src, dst) to flat
     row ids of the dense [N*N, D] per-graph image; fold the diagonal fill in
     as N extra tokens; pre-subtract the background vector v2 = emb_table[2]
     from every token payload (scatter is an ADD on top of the background);
     bucket tokens by 16K-row chunk and encode each token's chunk-local row
     r as idx = (r%128)<<8 | (r//128) for the SBUF-destination scatter.
  2. device, per core (out = 2 graphs = 32 MiB, 8 chunks of 4 MiB,
     double-buffered SBUF tiles, tile t = chunk k % 2):
     - vector (DVE): widen v2 across tile t (the chunk background).
     - gpsimd (SWDGE): dma_scatter_add the chunk's token deltas INTO the
       SBUF tile (sbuf_tokens_per_rank=128 parity mode; parity bit 0,
       out_ap_other aliased) - SBUF->SBUF via the SDMA CCE adders, no
       random HBM access.
     - sync+scalar (the two HWDGE queues): one dense 4 MiB write per
       composed chunk, alternating queues with the tiles.
     Cadence = max(write, scatter-DGE) per chunk; random token placement
     never hits HBM, so writes stream at full sequential bandwidth.
  3. host: stack per-core outputs to [16, 256, 256, 64].

A direct-scatter path (background writes + dma_scatter_add RMW into HBM)
is kept as _build_program for benchmarking; it measures ~5-15% slower
because the random 256 B CCE read-modify-writes add ~45 us of poorly-
behaved HBM traffic that cannot be hidden under the sequential writes.
"""

import os
from contextlib import ExitStack

import numpy as np

import concourse.bacc as bacc
from concourse import mybir
from concourse.bass_utils import run_bass_kernel_spmd
from concourse.bass_interp import get_hw_module
from concourse._compat import cdiv

B = 16
N = 256
D = 64
NCORES = 8
GPC = B // NCORES  # graphs per core = 2
NROWS_G = N * N  # 65536 rows per graph
NQ = 4  # SWDGE queues (ucode max); view v preps/triggers on queue v % NQ

# defaults for the graded path
QSPLIT = 2  # scatter views per graph (view span = NROWS_G/QSPLIT rows)
SORT_ROWS = False  # random row order measured faster than sorted on HW

# background widen tiers, cumulative ends (exclusive) in D-blocks of the
# chunk span: lets chunk-0 writes start while later tiers still widen
TIERS_BY_QSPLIT = {
    1: (16, 64, 160, 320, 512),
    2: (16, 64, 160, 256),
    4: (16, 48, 128),
    8: (16, 64),
}

LAST_EXEC_NS = None
LAST_RESULTS = None


def _pack_bucket(rows, deltas, cap, vspan):
    """Pack one (graph-slot, view) bucket for dma_scatter_add.

    rows: int array in [0, vspan) - target rows in the view.
    deltas: [len(rows), D] f32 payload minus background.
    cap: uniform token capacity (multiple of 128).

    Returns (src [128, (cap/128)*D] f32, idx [128, cap/16] int16).
    Padding tokens add 0.0 to a row unused by real tokens (safe under the
    SDMA read-modify-write with no same-row concurrency).
    """
    n = len(rows)
    assert n <= cap
    cols = cap // 128
    cols16 = cap // 16

    # find an unused row for the zero-delta padding tokens
    used = np.zeros(vspan, bool)
    used[rows] = True
    pad_row = int(np.argmin(used))  # first unused row (vspan >> n always)

    rows_p = np.full(cap, pad_row, np.int64)
    rows_p[:n] = rows
    deltas_p = np.zeros((cap, D), np.float32)
    deltas_p[:n] = deltas

    # src: token i lives at [i % 128, (i // 128)*D : ...]
    src = np.ascontiguousarray(
        deltas_p.reshape(cols, 128, D).transpose(1, 0, 2)
    ).reshape(128, cols * D)
    # idx: token i at [i % 16, i // 16], replicated to all 8 gpsimd groups
    idx2 = rows_p.astype(np.int16).reshape(cols16, 16)
    idx = np.ascontiguousarray(idx2.T)
    idx_rep = np.ascontiguousarray(np.tile(idx, (8, 1)))
    return src, idx_rep


def _route(edge_attr, emb_table, edge_index, batch_vec, qsplit=QSPLIT,
           sort=SORT_ROWS):
    """Host-side routing: per-core scatter buckets keyed (core, slot, q)."""
    src, dst = np.asarray(edge_index[0]), np.asarray(edge_index[1])
    batch_vec = np.asarray(batch_vec)
    edge_attr = np.asarray(edge_attr, dtype=np.float32)
    emb_table = np.asarray(emb_table, dtype=np.float32)
    vspan = NROWS_G // qsplit

    counts = np.bincount(batch_vec, minlength=B)
    starts = np.cumsum(counts) - counts
    g = batch_vec[src]
    ls = src - starts[g]
    ld = dst - starts[g]
    ok = (ls >= 0) & (ls < N) & (ld >= 0) & (ld < N)  # jax drops OOB scatters
    g, ls, ld = g[ok], ls[ok], ld[ok]
    ea = edge_attr[ok]

    v1, v2 = emb_table[1], emb_table[2]
    delta_e = ea - v2[None, :]
    delta_d = (v1 - v2)[None, :].repeat(N, axis=0)
    diag_rows = np.arange(N) * (N + 1)

    buckets = {}
    row = ls * N + ld
    for core in range(NCORES):
        for slot in range(GPC):
            gb = core * GPC + slot
            m = g == gb
            r_all = np.concatenate([row[m], diag_rows])
            d_all = np.concatenate([delta_e[m], delta_d], axis=0)
            uniq = np.unique(r_all)
            if len(uniq) < len(r_all):
                # duplicate rows would race in the concurrent CCE adds
                # (lost updates) - pre-sum them on host. No-op for the
                # distinct-pair inputs this model generates.
                uniq, inv = np.unique(r_all, return_inverse=True)
                acc = np.zeros((len(uniq), D), np.float32)
                np.add.at(acc, inv, d_all)
                r_all, d_all = uniq, acc
            if sort:
                # ascending rows -> the scatter drain walks HBM in address
                # order (row-buffer locality)
                order = np.argsort(r_all, kind="stable")
                r_all, d_all = r_all[order], d_all[order]
            for q in range(qsplit):
                hm = (r_all >= q * vspan) & (r_all < (q + 1) * vspan)
                buckets[(core, slot, q)] = (r_all[hm] - q * vspan, d_all[hm])

    cap = max(len(r) for r, _ in buckets.values())
    cap = cdiv(max(cap, 128), 128) * 128
    return buckets, cap, emb_table


def build_in_maps(buckets, cap, emb_np, qsplit=QSPLIT):
    vspan = NROWS_G // qsplit
    in_maps = []
    for core in range(NCORES):
        srcs, idxs = [], []
        for slot in range(GPC):
            for q in range(qsplit):
                src, idxr = _pack_bucket(*buckets[(core, slot, q)], cap, vspan)
                srcs.append(src)
                idxs.append(idxr)
        in_maps.append(
            {
                "emb": emb_np,
                "src": np.ascontiguousarray(np.concatenate(srcs, axis=1)),
                "idx": np.ascontiguousarray(np.concatenate(idxs, axis=1)),
            }
        )
    return in_maps


def _build_program(
    cap,
    reps=1,
    qsplit=QSPLIT,
    do_bg=True,
    do_scat=True,
    internal_out=False,
    use_trigger=True,
    bg_order="chunk",
):
    """Build the per-core program. reps>1 repeats the whole body (used only
    for benchmarking: rep r's background waits for rep r-1's scatters).
    do_bg/do_scat/internal_out are bench-only knobs: drop the background
    writes or the scatters, and keep the 32 MiB out tensor device-local
    (Internal) with a tiny dummy ExternalOutput so per-run host I/O is small.

    Semaphore discipline: waits only ever target a semaphore's FULL value at
    that point (per-DMA completions interleave across the 16 SDMA engines, so
    intermediate values can be mixtures of several DMAs).
    """
    vspan = NROWS_G // qsplit
    nview = GPC * qsplit
    tiers = TIERS_BY_QSPLIT[qsplit]
    cols = cap // 128
    cols16 = cap // 16
    bounds = [0] + [t * D for t in tiers]  # tier boundaries in elems

    nc = bacc.Bacc(
        "TRN2",
        target_bir_lowering=False,
        debug=False,
        num_devices=NCORES,
        # all prepared scatters must fit in the SWDGE rings at once
        # (nview/NQ pending per queue)
        dynamic_dma_scratch_size=65536,
        num_swdge_queues=NQ,
    )
    emb_t = nc.dram_tensor("emb", [3, D], mybir.dt.float32, kind="ExternalInput").ap()
    src_t = nc.dram_tensor(
        "src", [128, nview * cols * D], mybir.dt.float32, kind="ExternalInput"
    ).ap()
    idx_t = nc.dram_tensor(
        "idx", [128, nview * cols16], mybir.dt.int16, kind="ExternalInput"
    ).ap()
    out_kind = "Internal" if internal_out else "ExternalOutput"
    out_t = nc.dram_tensor(
        "out", [GPC * NROWS_G, D], mybir.dt.float32, kind=out_kind
    ).ap()
    dum_t = (
        nc.dram_tensor("dum", [1, 1], mybir.dt.float32, kind="ExternalOutput").ap()
        if internal_out
        else None
    )
    if internal_out:
        # unused input, shape-compatible with dum: lets the bench runner
        # chain k executions (dum_i -> chain_{i+1}) inside one jit call
        nc.dram_tensor("chain", [1, 1], mybir.dt.float32, kind="ExternalInput")
    # contiguous chunks: chunk c = out rows [c*vspan, (c+1)*vspan) == exactly
    # scatter view c; partition p holds vspan/128 consecutive rows
    out_chunks = out_t.rearrange(
        "(c p w) d -> c p (w d)", c=nview, p=128, w=vspan // 128
    )
    bg_width = (vspan // 128) * D  # one chunk's span per partition, in f32

    nc.reset()

    with (
        ExitStack() as stack,
        nc.sbuf_tensor([128, bg_width], mybir.dt.float32) as bg,
        nc.sbuf_tensor([128, nview * cols * D], mybir.dt.float32) as pay,
        nc.sbuf_tensor([128, nview * cols16], mybir.dt.int16) as idx,
        nc.semaphore() as s_load,
        nc.semaphore() as s_pay,
        nc.semaphore() as s_scat,
        nc.semaphore() as s_prep,
        nc.semaphore() as s_dum,
        nc.Block() as block,
    ):
        s_tier = [
            stack.enter_context(nc.semaphore(name=f"s_t{i}"))
            for i in range(len(tiers))
        ]
        s_bgc = [
            stack.enter_context(nc.semaphore(name=f"s_bgc{i}")) for i in range(nview)
        ]

        # chunk 0 is written in widen-tier pieces (16 incs each);
        # later chunks are single whole-chunk DMAs (16 incs each)
        if bg_order == "chunk":
            bgc_full = [16 * len(tiers)] + [16] * (nview - 1)
        else:
            bgc_full = [16 * len(tiers)] * nview

        def _wait_prev_rep(q, r):
            if r == 0:
                return
            if do_scat:
                q.wait_ge(s_scat, 16 * nview * r)
            else:
                for c in range(nview):
                    q.wait_ge(s_bgc[c], bgc_full[c] * r)

        @block.sync
        def _(sync):
            for r in range(reps):
                if do_bg:
                    # benchmark mode: previous rep's scatters must finish
                    # before overwriting their rows (and the bg/pay tiles)
                    _wait_prev_rep(sync, r)
                    # v2 row broadcast into all 128 partitions
                    sync.dma_start(
                        out=bg[:, 0:D], in_=emb_t[2:3, :].to_broadcast([128, D])
                    ).then_inc(s_load, 16)
                    if bg_order == "chunk":
                        # chunk 0: tier pieces start while tiers still widen
                        for t in range(len(tiers)):
                            lo, hi = bounds[t], bounds[t + 1]
                            sync.wait_ge(s_tier[t], r + 1)
                            sync.dma_start(
                                out=out_chunks[0][:, lo:hi], in_=bg[:, lo:hi]
                            ).then_inc(s_bgc[0], 16)
                        # later chunks: whole-chunk writes (widen fully done)
                        for c in range(1, nview):
                            sync.dma_start(
                                out=out_chunks[c][:, :], in_=bg[:, :]
                            ).then_inc(s_bgc[c], 16)
                    else:  # tier-major (original): all chunks per tier
                        for t in range(len(tiers)):
                            lo, hi = bounds[t], bounds[t + 1]
                            sync.wait_ge(s_tier[t], r + 1)
                            for c in range(nview):
                                sync.dma_start(
                                    out=out_chunks[c][:, lo:hi], in_=bg[:, lo:hi]
                                ).then_inc(s_bgc[c], 16)
            if internal_out:
                if do_scat:
                    sync.wait_ge(s_scat, 16 * nview * reps)
                elif do_bg:
                    for c in range(nview):
                        sync.wait_ge(s_bgc[c], bgc_full[c] * reps)
                src_dum = bg if do_bg else pay
                sync.dma_start(out=dum_t[:, :], in_=src_dum[0:1, 0:1]).then_inc(
                    s_dum, 16
                )

        if do_bg:

            @block.vector
            def _(vector):
                for r in range(reps):
                    vector.wait_ge(s_load, 16 * (r + 1))
                    v2blk = bg[:, 0:D].rearrange("p (x d) -> p x d", x=1)
                    prev = 1  # first tier's copy starts after the v2 block
                    for t in range(len(tiers)):
                        lo, hi = prev * D, bounds[t + 1]
                        vector.tensor_copy(
                            out=bg[:, lo:hi].rearrange("p (x d) -> p x d", d=D),
                            in_=v2blk.to_broadcast([128, tiers[t] - prev, D]),
                        ).then_inc(s_tier[t], 1)
                        prev = tiers[t]

        if do_scat:

            @block.gpsimd
            def _(gpsimd):
                for r in range(reps):
                    if r > 0:
                        # pay/idx tiles are read by the prev rep's scatters
                        gpsimd.wait_ge(s_scat, 16 * nview * r)
                    gpsimd.dma_start(out=pay[:], in_=src_t[:, :]).then_inc(s_pay, 16)
                    gpsimd.dma_start(out=idx[:], in_=idx_t[:, :]).then_inc(s_pay, 16)
                    gpsimd.wait_ge(s_pay, 32 * (r + 1))
                    if use_trigger:
                        # prepare all scatters now: Q7 descriptor generation
                        # runs while sync is still writing the background
                        for v in range(nview):
                            gpsimd.dma_scatter_add(
                                out_ap=out_t[v * vspan : (v + 1) * vspan, :],
                                in_ap=pay[
                                    :, v * cols * D : (v + 1) * cols * D
                                ].rearrange("p (c d) -> p c d", d=D),
                                idxs_ap=idx[:, v * cols16 : (v + 1) * cols16],
                                num_idxs=cap,
                                num_idxs_reg=cap,
                                elem_size=D,
                                prepare_only=True,
                                sem=s_scat,
                                queue_num=v % NQ,
                            ).then_inc(s_prep, 1)
                        gpsimd.wait_ge(s_prep, nview * (r + 1))
                        # fire scatter v as soon as chunk v's bg landed
                        # (per-queue FIFO: queue q holds preps q, q+NQ, ...,
                        # and is triggered in exactly that order)
                        for v in range(nview):
                            if do_bg:
                                gpsimd.wait_ge(s_bgc[v], bgc_full[v] * (r + 1))
                            gpsimd.trigger_dma(count=1, queue_num=v % NQ)
                    else:
                        for v in range(nview):
                            if do_bg:
                                gpsimd.wait_ge(s_bgc[v], bgc_full[v] * (r + 1))
                            gpsimd.dma_scatter_add(
                                out_ap=out_t[v * vspan : (v + 1) * vspan, :],
                                in_ap=pay[
                                    :, v * cols * D : (v + 1) * cols * D
                                ].rearrange("p (c d) -> p c d", d=D),
                                idxs_ap=idx[:, v * cols16 : (v + 1) * cols16],
                                num_idxs=cap,
                                num_idxs_reg=cap,
                                elem_size=D,
                            ).then_inc(s_scat, 16)
                # triggered drains must land before the program quiesces
                gpsimd.wait_ge(s_scat, 16 * nview * reps)

    nc.compile()
    nc.m = get_hw_module(nc.m)
    return nc


def _pack_bucket_compose(rows, deltas, cap, vspan):
    """Pack one chunk bucket for the SBUF-destination scatter.

    idx encoding for sbuf_tokens_per_rank=128 (dhi=1): dest partition =
    idx % 128, free-dim slot = idx >> 8, parity bit 7 always 0 (all tokens
    route to out_ap).  Chunk-local row r -> partition r//128, slot r%128:
        idx = (r % 128) << 8 | (r // 128)            (max 32639, int16 ok)
    Padding tokens add 0.0 to a row unused by real tokens (a concurrent
    CCE read-modify-write of a REAL token's row could lose that token's
    add, so zero-payload is not by itself safe).
    """
    n = len(rows)
    assert n <= cap and vspan == 16384
    cols = cap // 128
    cols16 = cap // 16

    used = np.zeros(vspan, bool)
    used[rows] = True
    pad_row = int(np.argmin(used))  # first unused row (vspan >> n always)

    enc = (rows % 128) << 8 | (rows // 128)
    pad_enc = (pad_row % 128) << 8 | (pad_row // 128)
    enc_p = np.full(cap, pad_enc, np.int64)
    enc_p[:n] = enc
    deltas_p = np.zeros((cap, D), np.float32)
    deltas_p[:n] = deltas

    src = np.ascontiguousarray(
        deltas_p.reshape(cols, 128, D).transpose(1, 0, 2)
    ).reshape(128, cols * D)
    idx2 = enc_p.astype(np.int16).reshape(cols16, 16)
    idx = np.ascontiguousarray(idx2.T)
    idx_rep = np.ascontiguousarray(np.tile(idx, (8, 1)))
    return src, idx_rep


def build_in_maps_compose(buckets, cap, emb_np):
    in_maps = []
    for core in range(NCORES):
        srcs, idxs = [], []
        for slot in range(GPC):
            for q in range(4):
                src, idxr = _pack_bucket_compose(
                    *buckets[(core, slot, q)], cap, NROWS_G // 4
                )
                srcs.append(src)
                idxs.append(idxr)
        in_maps.append(
            {
                "emb": emb_np,
                "src": np.ascontiguousarray(np.concatenate(srcs, axis=1)),
                "idx": np.ascontiguousarray(np.concatenate(idxs, axis=1)),
            }
        )
    return in_maps


def _build_program_compose(cap, reps=1, internal_out=False, two_queues=True):
    """SBUF-compose pipeline: never touches HBM randomly.

    Per chunk k (8 chunks of 16384 rows per rep; tile t = k % 2):
      vector (DVE): widen v2 into tile t              (after write k-2 done)
      gpsimd (SWDGE): dma_scatter_add token deltas INTO the SBUF tile
        (sbuf_tokens_per_rank=128 parity mode, parity bit 0, out_ap_other
        aliased to out_ap)                            (after widen k)
      sync (HWDGE): one dense 4 MiB write tile -> out chunk k
    HBM sees only the sequential chunk writes + the one-time payload loads;
    all token placement happens SBUF->SBUF through the SDMA CCE adders.
    """
    qsplit = 4
    vspan = NROWS_G // qsplit  # 16384
    nview = GPC * qsplit  # 8 chunks per core
    slots = vspan // 128  # 128 rows per partition per chunk
    cols = cap // 128
    cols16 = cap // 16

    nc = bacc.Bacc(
        "TRN2",
        target_bir_lowering=False,
        debug=False,
        num_devices=NCORES,
        dynamic_dma_scratch_size=65536,
    )
    emb_t = nc.dram_tensor("emb", [3, D], mybir.dt.float32, kind="ExternalInput").ap()
    src_t = nc.dram_tensor(
        "src", [128, nview * cols * D], mybir.dt.float32, kind="ExternalInput"
    ).ap()
    idx_t = nc.dram_tensor(
        "idx", [128, nview * cols16], mybir.dt.int16, kind="ExternalInput"
    ).ap()
    out_kind = "Internal" if internal_out else "ExternalOutput"
    out_t = nc.dram_tensor(
        "out", [GPC * NROWS_G, D], mybir.dt.float32, kind=out_kind
    ).ap()
    dum_t = (
        nc.dram_tensor("dum", [1, 1], mybir.dt.float32, kind="ExternalOutput").ap()
        if internal_out
        else None
    )
    if internal_out:
        nc.dram_tensor("chain", [1, 1], mybir.dt.float32, kind="ExternalInput")
    out_chunks = out_t.rearrange("(c p w) d -> c p (w d)", c=nview, p=128, w=slots)

    nc.reset()

    K = nview * reps  # global chunk counter

    with (
        nc.sbuf_tensor([128, slots * D], mybir.dt.float32) as tile0,
        nc.sbuf_tensor([128, slots * D], mybir.dt.float32) as tile1,
        nc.sbuf_tensor([128, D], mybir.dt.float32) as v2t,
        nc.sbuf_tensor([128, nview * cols * D], mybir.dt.float32) as pay,
        nc.sbuf_tensor([128, nview * cols16], mybir.dt.int16) as idx,
        nc.semaphore() as s_ld,
        nc.semaphore() as s_w0,
        nc.semaphore() as s_w1,
        nc.semaphore() as s_sc0,
        nc.semaphore() as s_sc1,
        nc.semaphore() as s_wr0,
        nc.semaphore() as s_wr1,
        nc.semaphore() as s_dum,
        nc.Block() as block,
    ):
        tiles = [tile0, tile1]
        s_wide = [s_w0, s_w1]
        s_sc = [s_sc0, s_sc1]
        s_wr = [s_wr0, s_wr1]

        @block.gpsimd
        def _(gpsimd):
            gpsimd.dma_start(
                out=v2t[:, :], in_=emb_t[2:3, :].to_broadcast([128, D])
            ).then_inc(s_ld, 16)
            gpsimd.dma_start(out=pay[:], in_=src_t[:, :]).then_inc(s_ld, 16)
            gpsimd.dma_start(out=idx[:], in_=idx_t[:, :]).then_inc(s_ld, 16)
            gpsimd.wait_ge(s_ld, 48)
            for k in range(K):
                t, c = k % 2, k % nview
                gpsimd.wait_ge(s_wide[t], k // 2 + 1)
                gpsimd.dma_scatter_add(
                    out_ap=tiles[t][:, :],
                    in_ap=pay[:, c * cols * D : (c + 1) * cols * D].rearrange(
                        "p (c d) -> p c d", d=D
                    ),
                    idxs_ap=idx[:, c * cols16 : (c + 1) * cols16],
                    num_idxs=cap,
                    num_idxs_reg=cap,
                    elem_size=D,
                    sbuf_tokens_per_rank=128,
                    parity_reg=0,
                    out_ap_other=tiles[t][:, :],
                ).then_inc(s_sc[t], 16)

        @block.vector
        def _(vector):
            vector.wait_ge(s_ld, 16)  # v2 loaded (first load on gpsimd queue)
            v2blk = v2t[:, :].rearrange("p (x d) -> p x d", x=1)
            for k in range(K):
                t = k % 2
                if k >= 2:
                    vector.wait_ge(s_wr[t], 16 * (k // 2))
                vector.tensor_copy(
                    out=tiles[t][:, :].rearrange("p (x d) -> p x d", d=D),
                    in_=v2blk.to_broadcast([128, slots, D]),
                ).then_inc(s_wide[t], 1)

        def _writer(q, parity):
            # writes for chunks k with k%2==parity (tile `parity`); sync and
            # scalar are both HWDGE queues, so the two tiles' writes overlap
            # their per-DMA completion latencies
            for k in range(parity, K, 2):
                c = k % nview
                q.wait_ge(s_sc[parity], 16 * (k // 2 + 1))
                q.dma_start(
                    out=out_chunks[c][:, :], in_=tiles[parity][:, :]
                ).then_inc(s_wr[parity], 16)

        @block.sync
        def _(sync):
            if two_queues:
                _writer(sync, 0)
            else:
                for k in range(K):
                    t, c = k % 2, k % nview
                    sync.wait_ge(s_sc[t], 16 * (k // 2 + 1))
                    sync.dma_start(
                        out=out_chunks[c][:, :], in_=tiles[t][:, :]
                    ).then_inc(s_wr[t], 16)
            if internal_out:
                sync.wait_ge(s_wr[0], 16 * ((K + 1) // 2))
                sync.wait_ge(s_wr[1], 16 * (K // 2))
                sync.dma_start(out=dum_t[:, :], in_=v2t[0:1, 0:1]).then_inc(s_dum, 16)

        if two_queues:

            @block.scalar
            def _(scalar):
                _writer(scalar, 1)

    nc.compile()
    nc.m = get_hw_module(nc.m)
    return nc


def prepare(edge_attr, emb_table, edge_index, batch_vec):
    """Host routing + program build (SBUF-compose path). Returns (nc, in_maps)."""
    buckets, cap, emb_np = _route(
        edge_attr, emb_table, edge_index, batch_vec, qsplit=4, sort=False
    )
    nc = _build_program_compose(cap)
    return nc, build_in_maps_compose(buckets, cap, emb_np)


def kernel(edge_attr, emb_table, edge_index, batch_vec):
    global LAST_EXEC_NS, LAST_RESULTS
    nc, in_maps = prepare(edge_attr, emb_table, edge_index, batch_vec)

    trace = bool(int(os.environ.get("BASSK_TRACE", "0")))
    res = run_bass_kernel_spmd(nc, in_maps, list(range(NCORES)), trace=trace)
    LAST_EXEC_NS = res.exec_time_ns
    LAST_RESULTS = res

    out = np.empty((B, N, N, D), np.float32)
    for core in range(NCORES):
        blockv = res.results[core]["out"].reshape(GPC, N, N, D)
        out[core * GPC : (core + 1) * GPC] = blockv
    return out


# revision 60
# speedup vs baseline: 1.1413x; 1.1007x over previous
"""Trainium2 Bass kernel for nn_DenseEdgeEncoder.

Computes, for B=16 graphs of N=256 nodes with 4096 edges each:
    out[b, i, j, :] = edge_attr[e]      if edge e = (i, j) in graph b
                      emb_table[1]      if i == j (self-loop fill)
                      emb_table[2]      otherwise
(the reference's scatter + embedding-lookup formulation reduces to this;
duplicate edges would scatter-add, which the delta-add below preserves).

Strategy (data-parallel over B, 2 graphs per core on 8 cores), the
"SBUF-compose" pipeline — HBM is only ever touched sequentially:
  1. host: route each graph's edges to its core; convert (src, dst) to flat
     row ids of the dense [N*N, D] per-graph image; fold the diagonal fill in
     as N extra tokens; pre-subtract the background vector v2 = emb_table[2]
     from every token payload (scatter is an ADD on top of the background);
     bucket tokens by 16K-row chunk and encode each token's chunk-local row
     r as idx = (r%128)<<8 | (r//128) for the SBUF-destination scatter.
  2. device, per core (out = 2 graphs = 32 MiB, 8 chunks of 4 MiB,
     double-buffered SBUF tiles, tile t = chunk k % 2):
     - vector (DVE): widen v2 across tile t (the chunk background).
     - gpsimd (SWDGE): dma_scatter_add the chunk's token deltas INTO the
       SBUF tile (sbuf_tokens_per_rank=128 parity mode; parity bit 0,
       out_ap_other aliased) - SBUF->SBUF via the SDMA CCE adders, no
       random HBM access.
     - sync+scalar (the two HWDGE queues): one dense 4 MiB write per
       composed chunk, alternating queues with the tiles.
     Cadence = max(write, scatter-DGE) per chunk; random token placement
     never hits HBM, so writes stream at full sequential bandwidth.
  3. host: stack per-core outputs to [16, 256, 256, 64].

A direct-scatter path (background writes + dma_scatter_add RMW into HBM)
is kept as _build_program for benchmarking; it measures ~5-15% slower
because the random 256 B CCE read-modify-writes add ~45 us of poorly-
behaved HBM traffic that cannot be hidden under the sequential writes.
"""

import os
from contextlib import ExitStack

import numpy as np

import concourse.bacc as bacc
from concourse import mybir
from concourse.bass_utils import run_bass_kernel_spmd
from concourse.bass_interp import get_hw_module
from concourse._compat import cdiv

B = 16
N = 256
D = 64
NCORES = 8
GPC = B // NCORES  # graphs per core = 2
NROWS_G = N * N  # 65536 rows per graph
NQ = 4  # SWDGE queues (ucode max); view v preps/triggers on queue v % NQ

# defaults for the graded path
QSPLIT = 2  # scatter views per graph (view span = NROWS_G/QSPLIT rows)
SORT_ROWS = False  # random row order measured faster than sorted on HW

# background widen tiers, cumulative ends (exclusive) in D-blocks of the
# chunk span: lets chunk-0 writes start while later tiers still widen
TIERS_BY_QSPLIT = {
    1: (16, 64, 160, 320, 512),
    2: (16, 64, 160, 256),
    4: (16, 48, 128),
    8: (16, 64),
}

LAST_EXEC_NS = None
LAST_RESULTS = None


def _pack_bucket(rows, deltas, cap, vspan):
    """Pack one (graph-slot, view) bucket for dma_scatter_add.

    rows: int array in [0, vspan) - target rows in the view.
    deltas: [len(rows), D] f32 payload minus background.
    cap: uniform token capacity (multiple of 128).

    Returns (src [128, (cap/128)*D] f32, idx [128, cap/16] int16).
    Padding tokens add 0.0 to a row unused by real tokens (safe under the
    SDMA read-modify-write with no same-row concurrency).
    """
    n = len(rows)
    assert n <= cap
    cols = cap // 128
    cols16 = cap // 16

    # find an unused row for the zero-delta padding tokens
    used = np.zeros(vspan, bool)
    used[rows] = True
    pad_row = int(np.argmin(used))  # first unused row (vspan >> n always)

    rows_p = np.full(cap, pad_row, np.int64)
    rows_p[:n] = rows
    deltas_p = np.zeros((cap, D), np.float32)
    deltas_p[:n] = deltas

    # src: token i lives at [i % 128, (i // 128)*D : ...]
    src = np.ascontiguousarray(
        deltas_p.reshape(cols, 128, D).transpose(1, 0, 2)
    ).reshape(128, cols * D)
    # idx: token i at [i % 16, i // 16], replicated to all 8 gpsimd groups
    idx2 = rows_p.astype(np.int16).reshape(cols16, 16)
    idx = np.ascontiguousarray(idx2.T)
    idx_rep = np.ascontiguousarray(np.tile(idx, (8, 1)))
    return src, idx_rep


def _route(edge_attr, emb_table, edge_index, batch_vec, qsplit=QSPLIT,
           sort=SORT_ROWS):
    """Host-side routing: per-core scatter buckets keyed (core, slot, q)."""
    src, dst = np.asarray(edge_index[0]), np.asarray(edge_index[1])
    batch_vec = np.asarray(batch_vec)
    edge_attr = np.asarray(edge_attr, dtype=np.float32)
    emb_table = np.asarray(emb_table, dtype=np.float32)
    vspan = NROWS_G // qsplit

    counts = np.bincount(batch_vec, minlength=B)
    starts = np.cumsum(counts) - counts
    g = batch_vec[src]
    ls = src - starts[g]
    ld = dst - starts[g]
    ok = (ls >= 0) & (ls < N) & (ld >= 0) & (ld < N)  # jax drops OOB scatters
    g, ls, ld = g[ok], ls[ok], ld[ok]
    ea = edge_attr[ok]

    v1, v2 = emb_table[1], emb_table[2]
    delta_e = ea - v2[None, :]
    delta_d = (v1 - v2)[None, :].repeat(N, axis=0)
    diag_rows = np.arange(N) * (N + 1)

    buckets = {}
    row = ls * N + ld
    for core in range(NCORES):
        for slot in range(GPC):
            gb = core * GPC + slot
            m = g == gb
            r_all = np.concatenate([row[m], diag_rows])
            d_all = np.concatenate([delta_e[m], delta_d], axis=0)
            uniq = np.unique(r_all)
            if len(uniq) < len(r_all):
                # duplicate rows would race in the concurrent CCE adds
                # (lost updates) - pre-sum them on host. No-op for the
                # distinct-pair inputs this model generates.
                uniq, inv = np.unique(r_all, return_inverse=True)
                acc = np.zeros((len(uniq), D), np.float32)
                np.add.at(acc, inv, d_all)
                r_all, d_all = uniq, acc
            if sort:
                # ascending rows -> the scatter drain walks HBM in address
                # order (row-buffer locality)
                order = np.argsort(r_all, kind="stable")
                r_all, d_all = r_all[order], d_all[order]
            for q in range(qsplit):
                hm = (r_all >= q * vspan) & (r_all < (q + 1) * vspan)
                buckets[(core, slot, q)] = (r_all[hm] - q * vspan, d_all[hm])

    cap = max(len(r) for r, _ in buckets.values())
    cap = cdiv(max(cap, 128), 128) * 128
    return buckets, cap, emb_table


def build_in_maps(buckets, cap, emb_np, qsplit=QSPLIT):
    vspan = NROWS_G // qsplit
    in_maps = []
    for core in range(NCORES):
        srcs, idxs = [], []
        for slot in range(GPC):
            for q in range(qsplit):
                src, idxr = _pack_bucket(*buckets[(core, slot, q)], cap, vspan)
                srcs.append(src)
                idxs.append(idxr)
        in_maps.append(
            {
                "emb": emb_np,
                "src": np.ascontiguousarray(np.concatenate(srcs, axis=1)),
                "idx": np.ascontiguousarray(np.concatenate(idxs, axis=1)),
            }
        )
    return in_maps


def _build_program(
    cap,
    reps=1,
    qsplit=QSPLIT,
    do_bg=True,
    do_scat=True,
    internal_out=False,
    use_trigger=True,
    bg_order="chunk",
):
    """Build the per-core program. reps>1 repeats the whole body (used only
    for benchmarking: rep r's background waits for rep r-1's scatters).
    do_bg/do_scat/internal_out are bench-only knobs: drop the background
    writes or the scatters, and keep the 32 MiB out tensor device-local
    (Internal) with a tiny dummy ExternalOutput so per-run host I/O is small.

    Semaphore discipline: waits only ever target a semaphore's FULL value at
    that point (per-DMA completions interleave across the 16 SDMA engines, so
    intermediate values can be mixtures of several DMAs).
    """
    vspan = NROWS_G // qsplit
    nview = GPC * qsplit
    tiers = TIERS_BY_QSPLIT[qsplit]
    cols = cap // 128
    cols16 = cap // 16
    bounds = [0] + [t * D for t in tiers]  # tier boundaries in elems

    nc = bacc.Bacc(
        "TRN2",
        target_bir_lowering=False,
        debug=False,
        num_devices=NCORES,
        # all prepared scatters must fit in the SWDGE rings at once
        # (nview/NQ pending per queue)
        dynamic_dma_scratch_size=65536,
        num_swdge_queues=NQ,
    )
    emb_t = nc.dram_tensor("emb", [3, D], mybir.dt.float32, kind="ExternalInput").ap()
    src_t = nc.dram_tensor(
        "src", [128, nview * cols * D], mybir.dt.float32, kind="ExternalInput"
    ).ap()
    idx_t = nc.dram_tensor(
        "idx", [128, nview * cols16], mybir.dt.int16, kind="ExternalInput"
    ).ap()
    out_kind = "Internal" if internal_out else "ExternalOutput"
    out_t = nc.dram_tensor(
        "out", [GPC * NROWS_G, D], mybir.dt.float32, kind=out_kind
    ).ap()
    dum_t = (
        nc.dram_tensor("dum", [1, 1], mybir.dt.float32, kind="ExternalOutput").ap()
        if internal_out
        else None
    )
    if internal_out:
        # unused input, shape-compatible with dum: lets the bench runner
        # chain k executions (dum_i -> chain_{i+1}) inside one jit call
        nc.dram_tensor("chain", [1, 1], mybir.dt.float32, kind="ExternalInput")
    # contiguous chunks: chunk c = out rows [c*vspan, (c+1)*vspan) == exactly
    # scatter view c; partition p holds vspan/128 consecutive rows
    out_chunks = out_t.rearrange(
        "(c p w) d -> c p (w d)", c=nview, p=128, w=vspan // 128
    )
    bg_width = (vspan // 128) * D  # one chunk's span per partition, in f32

    nc.reset()

    with (
        ExitStack() as stack,
        nc.sbuf_tensor([128, bg_width], mybir.dt.float32) as bg,
        nc.sbuf_tensor([128, nview * cols * D], mybir.dt.float32) as pay,
        nc.sbuf_tensor([128, nview * cols16], mybir.dt.int16) as idx,
        nc.semaphore() as s_load,
        nc.semaphore() as s_pay,
        nc.semaphore() as s_scat,
        nc.semaphore() as s_prep,
        nc.semaphore() as s_dum,
        nc.Block() as block,
    ):
        s_tier = [
            stack.enter_context(nc.semaphore(name=f"s_t{i}"))
            for i in range(len(tiers))
        ]
        s_bgc = [
            stack.enter_context(nc.semaphore(name=f"s_bgc{i}")) for i in range(nview)
        ]

        # chunk 0 is written in widen-tier pieces (16 incs each);
        # later chunks are single whole-chunk DMAs (16 incs each)
        if bg_order == "chunk":
            bgc_full = [16 * len(tiers)] + [16] * (nview - 1)
        else:
            bgc_full = [16 * len(tiers)] * nview

        def _wait_prev_rep(q, r):
            if r == 0:
                return
            if do_scat:
                q.wait_ge(s_scat, 16 * nview * r)
            else:
                for c in range(nview):
                    q.wait_ge(s_bgc[c], bgc_full[c] * r)

        @block.sync
        def _(sync):
            for r in range(reps):
                if do_bg:
                    # benchmark mode: previous rep's scatters must finish
                    # before overwriting their rows (and the bg/pay tiles)
                    _wait_prev_rep(sync, r)
                    # v2 row broadcast into all 128 partitions
                    sync.dma_start(
                        out=bg[:, 0:D], in_=emb_t[2:3, :].to_broadcast([128, D])
                    ).then_inc(s_load, 16)
                    if bg_order == "chunk":
                        # chunk 0: tier pieces start while tiers still widen
                        for t in range(len(tiers)):
                            lo, hi = bounds[t], bounds[t + 1]
                            sync.wait_ge(s_tier[t], r + 1)
                            sync.dma_start(
                                out=out_chunks[0][:, lo:hi], in_=bg[:, lo:hi]
                            ).then_inc(s_bgc[0], 16)
                        # later chunks: whole-chunk writes (widen fully done)
                        for c in range(1, nview):
                            sync.dma_start(
                                out=out_chunks[c][:, :], in_=bg[:, :]
                            ).then_inc(s_bgc[c], 16)
                    else:  # tier-major (original): all chunks per tier
                        for t in range(len(tiers)):
                            lo, hi = bounds[t], bounds[t + 1]
                            sync.wait_ge(s_tier[t], r + 1)
                            for c in range(nview):
                                sync.dma_start(
                                    out=out_chunks[c][:, lo:hi], in_=bg[:, lo:hi]
                                ).then_inc(s_bgc[c], 16)
            if internal_out:
                if do_scat:
                    sync.wait_ge(s_scat, 16 * nview * reps)
                elif do_bg:
                    for c in range(nview):
                        sync.wait_ge(s_bgc[c], bgc_full[c] * reps)
                src_dum = bg if do_bg else pay
                sync.dma_start(out=dum_t[:, :], in_=src_dum[0:1, 0:1]).then_inc(
                    s_dum, 16
                )

        if do_bg:

            @block.vector
            def _(vector):
                for r in range(reps):
                    vector.wait_ge(s_load, 16 * (r + 1))
                    v2blk = bg[:, 0:D].rearrange("p (x d) -> p x d", x=1)
                    prev = 1  # first tier's copy starts after the v2 block
                    for t in range(len(tiers)):
                        lo, hi = prev * D, bounds[t + 1]
                        vector.tensor_copy(
                            out=bg[:, lo:hi].rearrange("p (x d) -> p x d", d=D),
                            in_=v2blk.to_broadcast([128, tiers[t] - prev, D]),
                        ).then_inc(s_tier[t], 1)
                        prev = tiers[t]

        if do_scat:

            @block.gpsimd
            def _(gpsimd):
                for r in range(reps):
                    if r > 0:
                        # pay/idx tiles are read by the prev rep's scatters
                        gpsimd.wait_ge(s_scat, 16 * nview * r)
                    gpsimd.dma_start(out=pay[:], in_=src_t[:, :]).then_inc(s_pay, 16)
                    gpsimd.dma_start(out=idx[:], in_=idx_t[:, :]).then_inc(s_pay, 16)
                    gpsimd.wait_ge(s_pay, 32 * (r + 1))
                    if use_trigger:
                        # prepare all scatters now: Q7 descriptor generation
                        # runs while sync is still writing the background
                        for v in range(nview):
                            gpsimd.dma_scatter_add(
                                out_ap=out_t[v * vspan : (v + 1) * vspan, :],
                                in_ap=pay[
                                    :, v * cols * D : (v + 1) * cols * D
                                ].rearrange("p (c d) -> p c d", d=D),
                                idxs_ap=idx[:, v * cols16 : (v + 1) * cols16],
                                num_idxs=cap,
                                num_idxs_reg=cap,
                                elem_size=D,
                                prepare_only=True,
                                sem=s_scat,
                                queue_num=v % NQ,
                            ).then_inc(s_prep, 1)
                        gpsimd.wait_ge(s_prep, nview * (r + 1))
                        # fire scatter v as soon as chunk v's bg landed
                        # (per-queue FIFO: queue q holds preps q, q+NQ, ...,
                        # and is triggered in exactly that order)
                        for v in range(nview):
                            if do_bg:
                                gpsimd.wait_ge(s_bgc[v], bgc_full[v] * (r + 1))
                            gpsimd.trigger_dma(count=1, queue_num=v % NQ)
                    else:
                        for v in range(nview):
                            if do_bg:
                                gpsimd.wait_ge(s_bgc[v], bgc_full[v] * (r + 1))
                            gpsimd.dma_scatter_add(
                                out_ap=out_t[v * vspan : (v + 1) * vspan, :],
                                in_ap=pay[
                                    :, v * cols * D : (v + 1) * cols * D
                                ].rearrange("p (c d) -> p c d", d=D),
                                idxs_ap=idx[:, v * cols16 : (v + 1) * cols16],
                                num_idxs=cap,
                                num_idxs_reg=cap,
                                elem_size=D,
                            ).then_inc(s_scat, 16)
                # triggered drains must land before the program quiesces
                gpsimd.wait_ge(s_scat, 16 * nview * reps)

    nc.compile()
    nc.m = get_hw_module(nc.m)
    return nc


def _pack_bucket_compose(rows, deltas, cap, vspan):
    """Pack one chunk bucket for the SBUF-destination scatter.

    idx encoding for sbuf_tokens_per_rank=128 (dhi=1): dest partition =
    idx % 128, free-dim slot = idx >> 8, parity bit 7 always 0 (all tokens
    route to out_ap).  Chunk-local row r -> partition r//128, slot r%128:
        idx = (r % 128) << 8 | (r // 128)            (max 32639, int16 ok)
    Padding tokens add 0.0 to a row unused by real tokens (a concurrent
    CCE read-modify-write of a REAL token's row could lose that token's
    add, so zero-payload is not by itself safe).
    """
    n = len(rows)
    slots = vspan // 128  # rows per partition; needs slots <= 128
    assert n <= cap and slots <= 128
    cols = cap // 128
    cols16 = cap // 16

    used = np.zeros(vspan, bool)
    used[rows] = True
    pad_row = int(np.argmin(used))  # first unused row (vspan >> n always)

    enc = (rows % slots) << 8 | (rows // slots)
    pad_enc = (pad_row % slots) << 8 | (pad_row // slots)
    enc_p = np.full(cap, pad_enc, np.int64)
    enc_p[:n] = enc
    deltas_p = np.zeros((cap, D), np.float32)
    deltas_p[:n] = deltas

    src = np.ascontiguousarray(
        deltas_p.reshape(cols, 128, D).transpose(1, 0, 2)
    ).reshape(128, cols * D)
    idx2 = enc_p.astype(np.int16).reshape(cols16, 16)
    idx = np.ascontiguousarray(idx2.T)
    idx_rep = np.ascontiguousarray(np.tile(idx, (8, 1)))
    return src, idx_rep


def build_in_maps_compose(buckets, cap, emb_np, qsplit=4):
    in_maps = []
    for core in range(NCORES):
        srcs, idxs = [], []
        for slot in range(GPC):
            for q in range(qsplit):
                src, idxr = _pack_bucket_compose(
                    *buckets[(core, slot, q)], cap, NROWS_G // qsplit
                )
                srcs.append(src)
                idxs.append(idxr)
        in_maps.append(
            {
                "emb": emb_np,
                "src": np.ascontiguousarray(np.concatenate(srcs, axis=1)),
                "idx": np.ascontiguousarray(np.concatenate(idxs, axis=1)),
            }
        )
    return in_maps


def _build_program_compose(
    cap,
    reps=1,
    internal_out=False,
    two_queues=True,
    do_scat=True,
    do_wr=True,
    qsplit=4,
):
    """SBUF-compose pipeline: never touches HBM randomly.

    Per chunk k (8 chunks of 16384 rows per rep; tile t = k % 2):
      vector (DVE): widen v2 into tile t              (after write k-2 done)
      gpsimd (SWDGE): dma_scatter_add token deltas INTO the SBUF tile
        (sbuf_tokens_per_rank=128 parity mode, parity bit 0, out_ap_other
        aliased to out_ap)                            (after widen k)
      sync (HWDGE): one dense 4 MiB write tile -> out chunk k
    HBM sees only the sequential chunk writes + the one-time payload loads;
    all token placement happens SBUF->SBUF through the SDMA CCE adders.
    """
    vspan = NROWS_G // qsplit
    nview = GPC * qsplit  # chunks per core
    slots = vspan // 128  # rows per partition per chunk (<=128)
    cols = cap // 128
    cols16 = cap // 16

    nc = bacc.Bacc(
        "TRN2",
        target_bir_lowering=False,
        debug=False,
        num_devices=NCORES,
        dynamic_dma_scratch_size=65536,
    )
    emb_t = nc.dram_tensor("emb", [3, D], mybir.dt.float32, kind="ExternalInput").ap()
    src_t = nc.dram_tensor(
        "src", [128, nview * cols * D], mybir.dt.float32, kind="ExternalInput"
    ).ap()
    idx_t = nc.dram_tensor(
        "idx", [128, nview * cols16], mybir.dt.int16, kind="ExternalInput"
    ).ap()
    out_kind = "Internal" if internal_out else "ExternalOutput"
    out_t = nc.dram_tensor(
        "out", [GPC * NROWS_G, D], mybir.dt.float32, kind=out_kind
    ).ap()
    dum_t = (
        nc.dram_tensor("dum", [1, 1], mybir.dt.float32, kind="ExternalOutput").ap()
        if internal_out
        else None
    )
    if internal_out:
        nc.dram_tensor("chain", [1, 1], mybir.dt.float32, kind="ExternalInput")
    out_chunks = out_t.rearrange("(c p w) d -> c p (w d)", c=nview, p=128, w=slots)

    nc.reset()

    K = nview * reps  # global chunk counter

    # 3 tiles: the widen->scatter->write chain on N tiles has cadence
    # (sum of stages)/N; with 2 tiles that exceeds max(stage), with 3 the
    # per-engine stage times become the floor
    ntiles = 3

    with (
        nc.sbuf_tensor([128, D], mybir.dt.float32) as v2t,
        nc.sbuf_tensor([128, nview * cols * D], mybir.dt.float32) as pay,
        nc.sbuf_tensor([128, nview * cols16], mybir.dt.int16) as idx,
        nc.semaphore() as s_ld,
        nc.semaphore() as s_dum,
        ExitStack() as stack,
        nc.Block() as block,
    ):
        tiles = [
            stack.enter_context(
                nc.sbuf_tensor(f"tile{i}", [128, slots * D], mybir.dt.float32)
            )
            for i in range(ntiles)
        ]
        s_wide = [
            stack.enter_context(nc.semaphore(name=f"s_w{i}")) for i in range(ntiles)
        ]
        s_sc = [
            stack.enter_context(nc.semaphore(name=f"s_sc{i}")) for i in range(ntiles)
        ]
        s_wr = [
            stack.enter_context(nc.semaphore(name=f"s_wr{i}")) for i in range(ntiles)
        ]

        def _nwrites(t):  # chunks k in [0, K) with k % ntiles == t
            return (K - t + ntiles - 1) // ntiles

        @block.gpsimd
        def _(gpsimd):
            gpsimd.dma_start(
                out=v2t[:, :], in_=emb_t[2:3, :].to_broadcast([128, D])
            ).then_inc(s_ld, 16)
            gpsimd.dma_start(out=pay[:], in_=src_t[:, :]).then_inc(s_ld, 16)
            gpsimd.dma_start(out=idx[:], in_=idx_t[:, :]).then_inc(s_ld, 16)
            gpsimd.wait_ge(s_ld, 48)
            for k in range(K):
                if not do_scat:
                    break
                t, c = k % ntiles, k % nview
                gpsimd.wait_ge(s_wide[t], k // ntiles + 1)
                gpsimd.dma_scatter_add(
                    out_ap=tiles[t][:, :],
                    in_ap=pay[:, c * cols * D : (c + 1) * cols * D].rearrange(
                        "p (c d) -> p c d", d=D
                    ),
                    idxs_ap=idx[:, c * cols16 : (c + 1) * cols16],
                    num_idxs=cap,
                    num_idxs_reg=cap,
                    elem_size=D,
                    sbuf_tokens_per_rank=128,
                    parity_reg=0,
                    out_ap_other=tiles[t][:, :],
                ).then_inc(s_sc[t], 16)

        @block.vector
        def _(vector):
            vector.wait_ge(s_ld, 16)  # v2 loaded (first load on gpsimd queue)
            v2blk = v2t[:, :].rearrange("p (x d) -> p x d", x=1)
            for k in range(K):
                t = k % ntiles
                if k >= ntiles:
                    if do_wr:
                        vector.wait_ge(s_wr[t], 16 * (k // ntiles))
                    elif do_scat:
                        vector.wait_ge(s_sc[t], 16 * (k // ntiles))
                vector.tensor_copy(
                    out=tiles[t][:, :].rearrange("p (x d) -> p x d", d=D),
                    in_=v2blk.to_broadcast([128, slots, D]),
                ).then_inc(s_wide[t], 1)

        def _writer(q, parity):
            # writes for chunks k with k%2==parity; sync and scalar are both
            # HWDGE queues, so consecutive chunks' writes overlap their
            # per-DMA completion latencies (tile index runs mod ntiles)
            for k in range(parity, K, 2):
                t, c = k % ntiles, k % nview
                if do_scat:
                    q.wait_ge(s_sc[t], 16 * (k // ntiles + 1))
                else:
                    q.wait_ge(s_wide[t], k // ntiles + 1)
                q.dma_start(out=out_chunks[c][:, :], in_=tiles[t][:, :]).then_inc(
                    s_wr[t], 16
                )

        @block.sync
        def _(sync):
            if do_wr:
                if two_queues:
                    _writer(sync, 0)
                else:
                    for k in range(K):
                        t, c = k % ntiles, k % nview
                        sync.wait_ge(s_sc[t], 16 * (k // ntiles + 1))
                        sync.dma_start(
                            out=out_chunks[c][:, :], in_=tiles[t][:, :]
                        ).then_inc(s_wr[t], 16)
            if internal_out:
                for t in range(ntiles):
                    if do_wr:
                        sync.wait_ge(s_wr[t], 16 * _nwrites(t))
                    elif do_scat:
                        sync.wait_ge(s_sc[t], 16 * _nwrites(t))
                sync.dma_start(out=dum_t[:, :], in_=v2t[0:1, 0:1]).then_inc(s_dum, 16)

        if two_queues and do_wr:

            @block.scalar
            def _(scalar):
                _writer(scalar, 1)

    nc.compile()
    nc.m = get_hw_module(nc.m)
    return nc


def prepare(edge_attr, emb_table, edge_index, batch_vec):
    """Host routing + program build (SBUF-compose path). Returns (nc, in_maps)."""
    buckets, cap, emb_np = _route(
        edge_attr, emb_table, edge_index, batch_vec, qsplit=4, sort=False
    )
    nc = _build_program_compose(cap)
    return nc, build_in_maps_compose(buckets, cap, emb_np)


def kernel(edge_attr, emb_table, edge_index, batch_vec):
    global LAST_EXEC_NS, LAST_RESULTS
    nc, in_maps = prepare(edge_attr, emb_table, edge_index, batch_vec)

    trace = bool(int(os.environ.get("BASSK_TRACE", "0")))
    res = run_bass_kernel_spmd(nc, in_maps, list(range(NCORES)), trace=trace)
    LAST_EXEC_NS = res.exec_time_ns
    LAST_RESULTS = res

    out = np.empty((B, N, N, D), np.float32)
    for core in range(NCORES):
        blockv = res.results[core]["out"].reshape(GPC, N, N, D)
        out[core * GPC : (core + 1) * GPC] = blockv
    return out


# revision 64
# speedup vs baseline: 1.1979x; 1.0496x over previous
"""Trainium2 Bass kernel for nn_DenseEdgeEncoder.

Computes, for B=16 graphs of N=256 nodes with 4096 edges each:
    out[b, i, j, :] = edge_attr[e]      if edge e = (i, j) in graph b
                      emb_table[1]      if i == j (self-loop fill)
                      emb_table[2]      otherwise
(the reference's scatter + embedding-lookup formulation reduces to this;
duplicate edges would scatter-add, which the delta-add below preserves).

Strategy (data-parallel over B, 2 graphs per core on 8 cores), the
"SBUF-compose" pipeline — HBM is only ever touched sequentially:
  1. host: route each graph's edges to its core; convert (src, dst) to flat
     row ids of the dense [N*N, D] per-graph image; fold the diagonal fill in
     as N extra tokens; pre-subtract the background vector v2 = emb_table[2]
     from every token payload (scatter is an ADD on top of the background);
     bucket tokens by 16K-row chunk and encode each token's chunk-local row
     r as idx = (r%128)<<8 | (r//128) for the SBUF-destination scatter.
  2. device, per core (out = 2 graphs = 32 MiB, 8 chunks of 4 MiB,
     double-buffered SBUF tiles, tile t = chunk k % 2):
     - vector (DVE): widen v2 across tile t (the chunk background).
     - gpsimd (SWDGE): dma_scatter_add the chunk's token deltas INTO the
       SBUF tile (sbuf_tokens_per_rank=128 parity mode; parity bit 0,
       out_ap_other aliased) - SBUF->SBUF via the SDMA CCE adders, no
       random HBM access.
     - sync+scalar (the two HWDGE queues): one dense 4 MiB write per
       composed chunk, alternating queues with the tiles.
     Cadence = max(write, scatter-DGE) per chunk; random token placement
     never hits HBM, so writes stream at full sequential bandwidth.
  3. host: stack per-core outputs to [16, 256, 256, 64].

A direct-scatter path (background writes + dma_scatter_add RMW into HBM)
is kept as _build_program for benchmarking; it measures ~5-15% slower
because the random 256 B CCE read-modify-writes add ~45 us of poorly-
behaved HBM traffic that cannot be hidden under the sequential writes.
"""

import os
from contextlib import ExitStack

import numpy as np

import concourse.bacc as bacc
from concourse import mybir
from concourse.bass_utils import run_bass_kernel_spmd
from concourse.bass_interp import get_hw_module
from concourse._compat import cdiv

B = 16
N = 256
D = 64
NCORES = 8
GPC = B // NCORES  # graphs per core = 2
NROWS_G = N * N  # 65536 rows per graph
NQ = 4  # SWDGE queues (ucode max); view v preps/triggers on queue v % NQ

# defaults for the graded path
QSPLIT = 2  # scatter views per graph (view span = NROWS_G/QSPLIT rows)
SORT_ROWS = False  # random row order measured faster than sorted on HW

# background widen tiers, cumulative ends (exclusive) in D-blocks of the
# chunk span: lets chunk-0 writes start while later tiers still widen
TIERS_BY_QSPLIT = {
    1: (16, 64, 160, 320, 512),
    2: (16, 64, 160, 256),
    4: (16, 48, 128),
    8: (16, 64),
}

LAST_EXEC_NS = None
LAST_RESULTS = None


def _pack_bucket(rows, deltas, cap, vspan):
    """Pack one (graph-slot, view) bucket for dma_scatter_add.

    rows: int array in [0, vspan) - target rows in the view.
    deltas: [len(rows), D] f32 payload minus background.
    cap: uniform token capacity (multiple of 128).

    Returns (src [128, (cap/128)*D] f32, idx [128, cap/16] int16).
    Padding tokens add 0.0 to a row unused by real tokens (safe under the
    SDMA read-modify-write with no same-row concurrency).
    """
    n = len(rows)
    assert n <= cap
    cols = cap // 128
    cols16 = cap // 16

    # find an unused row for the zero-delta padding tokens
    used = np.zeros(vspan, bool)
    used[rows] = True
    pad_row = int(np.argmin(used))  # first unused row (vspan >> n always)

    rows_p = np.full(cap, pad_row, np.int64)
    rows_p[:n] = rows
    deltas_p = np.zeros((cap, D), np.float32)
    deltas_p[:n] = deltas

    # src: token i lives at [i % 128, (i // 128)*D : ...]
    src = np.ascontiguousarray(
        deltas_p.reshape(cols, 128, D).transpose(1, 0, 2)
    ).reshape(128, cols * D)
    # idx: token i at [i % 16, i // 16], replicated to all 8 gpsimd groups
    idx2 = rows_p.astype(np.int16).reshape(cols16, 16)
    idx = np.ascontiguousarray(idx2.T)
    idx_rep = np.ascontiguousarray(np.tile(idx, (8, 1)))
    return src, idx_rep


def _route(edge_attr, emb_table, edge_index, batch_vec, qsplit=QSPLIT,
           sort=SORT_ROWS):
    """Host-side routing: per-core scatter buckets keyed (core, slot, q)."""
    src, dst = np.asarray(edge_index[0]), np.asarray(edge_index[1])
    batch_vec = np.asarray(batch_vec)
    edge_attr = np.asarray(edge_attr, dtype=np.float32)
    emb_table = np.asarray(emb_table, dtype=np.float32)
    vspan = NROWS_G // qsplit

    counts = np.bincount(batch_vec, minlength=B)
    starts = np.cumsum(counts) - counts
    g = batch_vec[src]
    ls = src - starts[g]
    ld = dst - starts[g]
    ok = (ls >= 0) & (ls < N) & (ld >= 0) & (ld < N)  # jax drops OOB scatters
    g, ls, ld = g[ok], ls[ok], ld[ok]
    ea = edge_attr[ok]

    v1, v2 = emb_table[1], emb_table[2]
    delta_e = ea - v2[None, :]
    delta_d = (v1 - v2)[None, :].repeat(N, axis=0)
    diag_rows = np.arange(N) * (N + 1)

    buckets = {}
    row = ls * N + ld
    for core in range(NCORES):
        for slot in range(GPC):
            gb = core * GPC + slot
            m = g == gb
            r_all = np.concatenate([row[m], diag_rows])
            d_all = np.concatenate([delta_e[m], delta_d], axis=0)
            uniq = np.unique(r_all)
            if len(uniq) < len(r_all):
                # duplicate rows would race in the concurrent CCE adds
                # (lost updates) - pre-sum them on host. No-op for the
                # distinct-pair inputs this model generates.
                uniq, inv = np.unique(r_all, return_inverse=True)
                acc = np.zeros((len(uniq), D), np.float32)
                np.add.at(acc, inv, d_all)
                r_all, d_all = uniq, acc
            if sort:
                # ascending rows -> the scatter drain walks HBM in address
                # order (row-buffer locality)
                order = np.argsort(r_all, kind="stable")
                r_all, d_all = r_all[order], d_all[order]
            for q in range(qsplit):
                hm = (r_all >= q * vspan) & (r_all < (q + 1) * vspan)
                buckets[(core, slot, q)] = (r_all[hm] - q * vspan, d_all[hm])

    cap = max(len(r) for r, _ in buckets.values())
    cap = cdiv(max(cap, 128), 128) * 128
    return buckets, cap, emb_table


def build_in_maps(buckets, cap, emb_np, qsplit=QSPLIT):
    vspan = NROWS_G // qsplit
    in_maps = []
    for core in range(NCORES):
        srcs, idxs = [], []
        for slot in range(GPC):
            for q in range(qsplit):
                src, idxr = _pack_bucket(*buckets[(core, slot, q)], cap, vspan)
                srcs.append(src)
                idxs.append(idxr)
        in_maps.append(
            {
                "emb": emb_np,
                "src": np.ascontiguousarray(np.concatenate(srcs, axis=1)),
                "idx": np.ascontiguousarray(np.concatenate(idxs, axis=1)),
            }
        )
    return in_maps


def _build_program(
    cap,
    reps=1,
    qsplit=QSPLIT,
    do_bg=True,
    do_scat=True,
    internal_out=False,
    use_trigger=True,
    bg_order="chunk",
):
    """Build the per-core program. reps>1 repeats the whole body (used only
    for benchmarking: rep r's background waits for rep r-1's scatters).
    do_bg/do_scat/internal_out are bench-only knobs: drop the background
    writes or the scatters, and keep the 32 MiB out tensor device-local
    (Internal) with a tiny dummy ExternalOutput so per-run host I/O is small.

    Semaphore discipline: waits only ever target a semaphore's FULL value at
    that point (per-DMA completions interleave across the 16 SDMA engines, so
    intermediate values can be mixtures of several DMAs).
    """
    vspan = NROWS_G // qsplit
    nview = GPC * qsplit
    tiers = TIERS_BY_QSPLIT[qsplit]
    cols = cap // 128
    cols16 = cap // 16
    bounds = [0] + [t * D for t in tiers]  # tier boundaries in elems

    nc = bacc.Bacc(
        "TRN2",
        target_bir_lowering=False,
        debug=False,
        num_devices=NCORES,
        # all prepared scatters must fit in the SWDGE rings at once
        # (nview/NQ pending per queue)
        dynamic_dma_scratch_size=65536,
        num_swdge_queues=NQ,
    )
    emb_t = nc.dram_tensor("emb", [3, D], mybir.dt.float32, kind="ExternalInput").ap()
    src_t = nc.dram_tensor(
        "src", [128, nview * cols * D], mybir.dt.float32, kind="ExternalInput"
    ).ap()
    idx_t = nc.dram_tensor(
        "idx", [128, nview * cols16], mybir.dt.int16, kind="ExternalInput"
    ).ap()
    out_kind = "Internal" if internal_out else "ExternalOutput"
    out_t = nc.dram_tensor(
        "out", [GPC * NROWS_G, D], mybir.dt.float32, kind=out_kind
    ).ap()
    dum_t = (
        nc.dram_tensor("dum", [1, 1], mybir.dt.float32, kind="ExternalOutput").ap()
        if internal_out
        else None
    )
    if internal_out:
        # unused input, shape-compatible with dum: lets the bench runner
        # chain k executions (dum_i -> chain_{i+1}) inside one jit call
        nc.dram_tensor("chain", [1, 1], mybir.dt.float32, kind="ExternalInput")
    # contiguous chunks: chunk c = out rows [c*vspan, (c+1)*vspan) == exactly
    # scatter view c; partition p holds vspan/128 consecutive rows
    out_chunks = out_t.rearrange(
        "(c p w) d -> c p (w d)", c=nview, p=128, w=vspan // 128
    )
    bg_width = (vspan // 128) * D  # one chunk's span per partition, in f32

    nc.reset()

    with (
        ExitStack() as stack,
        nc.sbuf_tensor([128, bg_width], mybir.dt.float32) as bg,
        nc.sbuf_tensor([128, nview * cols * D], mybir.dt.float32) as pay,
        nc.sbuf_tensor([128, nview * cols16], mybir.dt.int16) as idx,
        nc.semaphore() as s_load,
        nc.semaphore() as s_pay,
        nc.semaphore() as s_scat,
        nc.semaphore() as s_prep,
        nc.semaphore() as s_dum,
        nc.Block() as block,
    ):
        s_tier = [
            stack.enter_context(nc.semaphore(name=f"s_t{i}"))
            for i in range(len(tiers))
        ]
        s_bgc = [
            stack.enter_context(nc.semaphore(name=f"s_bgc{i}")) for i in range(nview)
        ]

        # chunk 0 is written in widen-tier pieces (16 incs each);
        # later chunks are single whole-chunk DMAs (16 incs each)
        if bg_order == "chunk":
            bgc_full = [16 * len(tiers)] + [16] * (nview - 1)
        else:
            bgc_full = [16 * len(tiers)] * nview

        def _wait_prev_rep(q, r):
            if r == 0:
                return
            if do_scat:
                q.wait_ge(s_scat, 16 * nview * r)
            else:
                for c in range(nview):
                    q.wait_ge(s_bgc[c], bgc_full[c] * r)

        @block.sync
        def _(sync):
            for r in range(reps):
                if do_bg:
                    # benchmark mode: previous rep's scatters must finish
                    # before overwriting their rows (and the bg/pay tiles)
                    _wait_prev_rep(sync, r)
                    # v2 row broadcast into all 128 partitions
                    sync.dma_start(
                        out=bg[:, 0:D], in_=emb_t[2:3, :].to_broadcast([128, D])
                    ).then_inc(s_load, 16)
                    if bg_order == "chunk":
                        # chunk 0: tier pieces start while tiers still widen
                        for t in range(len(tiers)):
                            lo, hi = bounds[t], bounds[t + 1]
                            sync.wait_ge(s_tier[t], r + 1)
                            sync.dma_start(
                                out=out_chunks[0][:, lo:hi], in_=bg[:, lo:hi]
                            ).then_inc(s_bgc[0], 16)
                        # later chunks: whole-chunk writes (widen fully done)
                        for c in range(1, nview):
                            sync.dma_start(
                                out=out_chunks[c][:, :], in_=bg[:, :]
                            ).then_inc(s_bgc[c], 16)
                    else:  # tier-major (original): all chunks per tier
                        for t in range(len(tiers)):
                            lo, hi = bounds[t], bounds[t + 1]
                            sync.wait_ge(s_tier[t], r + 1)
                            for c in range(nview):
                                sync.dma_start(
                                    out=out_chunks[c][:, lo:hi], in_=bg[:, lo:hi]
                                ).then_inc(s_bgc[c], 16)
            if internal_out:
                if do_scat:
                    sync.wait_ge(s_scat, 16 * nview * reps)
                elif do_bg:
                    for c in range(nview):
                        sync.wait_ge(s_bgc[c], bgc_full[c] * reps)
                src_dum = bg if do_bg else pay
                sync.dma_start(out=dum_t[:, :], in_=src_dum[0:1, 0:1]).then_inc(
                    s_dum, 16
                )

        if do_bg:

            @block.vector
            def _(vector):
                for r in range(reps):
                    vector.wait_ge(s_load, 16 * (r + 1))
                    v2blk = bg[:, 0:D].rearrange("p (x d) -> p x d", x=1)
                    prev = 1  # first tier's copy starts after the v2 block
                    for t in range(len(tiers)):
                        lo, hi = prev * D, bounds[t + 1]
                        vector.tensor_copy(
                            out=bg[:, lo:hi].rearrange("p (x d) -> p x d", d=D),
                            in_=v2blk.to_broadcast([128, tiers[t] - prev, D]),
                        ).then_inc(s_tier[t], 1)
                        prev = tiers[t]

        if do_scat:

            @block.gpsimd
            def _(gpsimd):
                for r in range(reps):
                    if r > 0:
                        # pay/idx tiles are read by the prev rep's scatters
                        gpsimd.wait_ge(s_scat, 16 * nview * r)
                    gpsimd.dma_start(out=pay[:], in_=src_t[:, :]).then_inc(s_pay, 16)
                    gpsimd.dma_start(out=idx[:], in_=idx_t[:, :]).then_inc(s_pay, 16)
                    gpsimd.wait_ge(s_pay, 32 * (r + 1))
                    if use_trigger:
                        # prepare all scatters now: Q7 descriptor generation
                        # runs while sync is still writing the background
                        for v in range(nview):
                            gpsimd.dma_scatter_add(
                                out_ap=out_t[v * vspan : (v + 1) * vspan, :],
                                in_ap=pay[
                                    :, v * cols * D : (v + 1) * cols * D
                                ].rearrange("p (c d) -> p c d", d=D),
                                idxs_ap=idx[:, v * cols16 : (v + 1) * cols16],
                                num_idxs=cap,
                                num_idxs_reg=cap,
                                elem_size=D,
                                prepare_only=True,
                                sem=s_scat,
                                queue_num=v % NQ,
                            ).then_inc(s_prep, 1)
                        gpsimd.wait_ge(s_prep, nview * (r + 1))
                        # fire scatter v as soon as chunk v's bg landed
                        # (per-queue FIFO: queue q holds preps q, q+NQ, ...,
                        # and is triggered in exactly that order)
                        for v in range(nview):
                            if do_bg:
                                gpsimd.wait_ge(s_bgc[v], bgc_full[v] * (r + 1))
                            gpsimd.trigger_dma(count=1, queue_num=v % NQ)
                    else:
                        for v in range(nview):
                            if do_bg:
                                gpsimd.wait_ge(s_bgc[v], bgc_full[v] * (r + 1))
                            gpsimd.dma_scatter_add(
                                out_ap=out_t[v * vspan : (v + 1) * vspan, :],
                                in_ap=pay[
                                    :, v * cols * D : (v + 1) * cols * D
                                ].rearrange("p (c d) -> p c d", d=D),
                                idxs_ap=idx[:, v * cols16 : (v + 1) * cols16],
                                num_idxs=cap,
                                num_idxs_reg=cap,
                                elem_size=D,
                            ).then_inc(s_scat, 16)
                # triggered drains must land before the program quiesces
                gpsimd.wait_ge(s_scat, 16 * nview * reps)

    nc.compile()
    nc.m = get_hw_module(nc.m)
    return nc


def _pack_bucket_compose(rows, deltas, cap, vspan):
    """Pack one chunk bucket for the SBUF-destination scatter.

    idx encoding for sbuf_tokens_per_rank=128 (dhi=1): dest partition =
    idx % 128, free-dim slot = idx >> 8, parity bit 7 always 0 (all tokens
    route to out_ap).  Chunk-local row r -> partition r//128, slot r%128:
        idx = (r % 128) << 8 | (r // 128)            (max 32639, int16 ok)
    Padding tokens add 0.0 to a row unused by real tokens (a concurrent
    CCE read-modify-write of a REAL token's row could lose that token's
    add, so zero-payload is not by itself safe).
    """
    n = len(rows)
    slots = vspan // 128  # rows per partition; needs slots <= 128
    assert n <= cap and slots <= 128
    cols = cap // 128
    cols16 = cap // 16

    used = np.zeros(vspan, bool)
    used[rows] = True
    pad_row = int(np.argmin(used))  # first unused row (vspan >> n always)

    enc = (rows % slots) << 8 | (rows // slots)
    pad_enc = (pad_row % slots) << 8 | (pad_row // slots)
    enc_p = np.full(cap, pad_enc, np.int64)
    enc_p[:n] = enc
    deltas_p = np.zeros((cap, D), np.float32)
    deltas_p[:n] = deltas

    src = np.ascontiguousarray(
        deltas_p.reshape(cols, 128, D).transpose(1, 0, 2)
    ).reshape(128, cols * D)
    idx2 = enc_p.astype(np.int16).reshape(cols16, 16)
    idx = np.ascontiguousarray(idx2.T)
    idx_rep = np.ascontiguousarray(np.tile(idx, (8, 1)))
    return src, idx_rep


def build_in_maps_compose(buckets, cap, emb_np, qsplit=4):
    in_maps = []
    for core in range(NCORES):
        srcs, idxs = [], []
        for slot in range(GPC):
            for q in range(qsplit):
                src, idxr = _pack_bucket_compose(
                    *buckets[(core, slot, q)], cap, NROWS_G // qsplit
                )
                srcs.append(src)
                idxs.append(idxr)
        in_maps.append(
            {
                "emb": emb_np,
                "src": np.ascontiguousarray(np.concatenate(srcs, axis=1)),
                "idx": np.ascontiguousarray(np.concatenate(idxs, axis=1)),
            }
        )
    return in_maps


def _build_program_compose(
    cap,
    reps=1,
    internal_out=False,
    two_queues=True,
    do_scat=True,
    do_wr=True,
    qsplit=4,
    ntiles=4,
    single_packet=True,
):
    """SBUF-compose pipeline: never touches HBM randomly.

    Per chunk k (8 chunks of 16384 rows per rep; tile t = k % 2):
      vector (DVE): widen v2 into tile t              (after write k-2 done)
      gpsimd (SWDGE): dma_scatter_add token deltas INTO the SBUF tile
        (sbuf_tokens_per_rank=128 parity mode, parity bit 0, out_ap_other
        aliased to out_ap)                            (after widen k)
      sync (HWDGE): one dense 4 MiB write tile -> out chunk k
    HBM sees only the sequential chunk writes + the one-time payload loads;
    all token placement happens SBUF->SBUF through the SDMA CCE adders.
    """
    vspan = NROWS_G // qsplit
    nview = GPC * qsplit  # chunks per core
    slots = vspan // 128  # rows per partition per chunk (<=128)
    cols = cap // 128
    cols16 = cap // 16

    nc = bacc.Bacc(
        "TRN2",
        target_bir_lowering=False,
        debug=False,
        num_devices=NCORES,
        dynamic_dma_scratch_size=65536,
    )
    emb_t = nc.dram_tensor("emb", [3, D], mybir.dt.float32, kind="ExternalInput").ap()
    src_t = nc.dram_tensor(
        "src", [128, nview * cols * D], mybir.dt.float32, kind="ExternalInput"
    ).ap()
    idx_t = nc.dram_tensor(
        "idx", [128, nview * cols16], mybir.dt.int16, kind="ExternalInput"
    ).ap()
    out_kind = "Internal" if internal_out else "ExternalOutput"
    out_t = nc.dram_tensor(
        "out", [GPC * NROWS_G, D], mybir.dt.float32, kind=out_kind
    ).ap()
    dum_t = (
        nc.dram_tensor("dum", [1, 1], mybir.dt.float32, kind="ExternalOutput").ap()
        if internal_out
        else None
    )
    if internal_out:
        nc.dram_tensor("chain", [1, 1], mybir.dt.float32, kind="ExternalInput")
    out_chunks = out_t.rearrange("(c p w) d -> c p (w d)", c=nview, p=128, w=slots)

    nc.reset()

    K = nview * reps  # global chunk counter

    # >=3 tiles: the widen->scatter->write chain on N tiles has cadence
    # (sum of stages)/N; with 2 tiles that exceeds max(stage), with 3 the
    # per-engine stage times become the floor
    with (
        nc.sbuf_tensor([128, D], mybir.dt.float32) as v2t,
        nc.sbuf_tensor([128, nview * cols * D], mybir.dt.float32) as pay,
        nc.sbuf_tensor([128, nview * cols16], mybir.dt.int16) as idx,
        nc.semaphore() as s_ld,
        nc.semaphore() as s_dum,
        ExitStack() as stack,
        nc.Block() as block,
    ):
        tiles = [
            stack.enter_context(
                nc.sbuf_tensor(f"tile{i}", [128, slots * D], mybir.dt.float32)
            )
            for i in range(ntiles)
        ]
        s_wide = [
            stack.enter_context(nc.semaphore(name=f"s_w{i}")) for i in range(ntiles)
        ]
        s_sc = [
            stack.enter_context(nc.semaphore(name=f"s_sc{i}")) for i in range(ntiles)
        ]
        s_wr = [
            stack.enter_context(nc.semaphore(name=f"s_wr{i}")) for i in range(ntiles)
        ]

        def _nwrites(t):  # chunks k in [0, K) with k % ntiles == t
            return (K - t + ntiles - 1) // ntiles

        @block.gpsimd
        def _(gpsimd):
            gpsimd.dma_start(
                out=v2t[:, :], in_=emb_t[2:3, :].to_broadcast([128, D])
            ).then_inc(s_ld, 16)
            gpsimd.dma_start(out=pay[:], in_=src_t[:, :]).then_inc(s_ld, 16)
            gpsimd.dma_start(out=idx[:], in_=idx_t[:, :]).then_inc(s_ld, 16)
            gpsimd.wait_ge(s_ld, 48)
            for k in range(K):
                if not do_scat:
                    break
                t, c = k % ntiles, k % nview
                gpsimd.wait_ge(s_wide[t], k // ntiles + 1)
                gpsimd.dma_scatter_add(
                    out_ap=tiles[t][:, :],
                    in_ap=pay[:, c * cols * D : (c + 1) * cols * D].rearrange(
                        "p (c d) -> p c d", d=D
                    ),
                    idxs_ap=idx[:, c * cols16 : (c + 1) * cols16],
                    num_idxs=cap,
                    num_idxs_reg=cap,
                    elem_size=D,
                    sbuf_tokens_per_rank=128,
                    parity_reg=0,
                    out_ap_other=tiles[t][:, :],
                    single_packet=single_packet,
                ).then_inc(s_sc[t], 16)

        @block.vector
        def _(vector):
            vector.wait_ge(s_ld, 16)  # v2 loaded (first load on gpsimd queue)
            v2blk = v2t[:, :].rearrange("p (x d) -> p x d", x=1)
            for k in range(K):
                t = k % ntiles
                if k >= ntiles:
                    if do_wr:
                        vector.wait_ge(s_wr[t], 16 * (k // ntiles))
                    elif do_scat:
                        vector.wait_ge(s_sc[t], 16 * (k // ntiles))
                vector.tensor_copy(
                    out=tiles[t][:, :].rearrange("p (x d) -> p x d", d=D),
                    in_=v2blk.to_broadcast([128, slots, D]),
                ).then_inc(s_wide[t], 1)

        def _writer(q, parity):
            # writes for chunks k with k%2==parity; sync and scalar are both
            # HWDGE queues, so consecutive chunks' writes overlap their
            # per-DMA completion latencies (tile index runs mod ntiles)
            for k in range(parity, K, 2):
                t, c = k % ntiles, k % nview
                if do_scat:
                    q.wait_ge(s_sc[t], 16 * (k // ntiles + 1))
                else:
                    q.wait_ge(s_wide[t], k // ntiles + 1)
                q.dma_start(out=out_chunks[c][:, :], in_=tiles[t][:, :]).then_inc(
                    s_wr[t], 16
                )

        @block.sync
        def _(sync):
            if do_wr:
                if two_queues:
                    _writer(sync, 0)
                else:
                    for k in range(K):
                        t, c = k % ntiles, k % nview
                        sync.wait_ge(s_sc[t], 16 * (k // ntiles + 1))
                        sync.dma_start(
                            out=out_chunks[c][:, :], in_=tiles[t][:, :]
                        ).then_inc(s_wr[t], 16)
            if internal_out:
                for t in range(ntiles):
                    if do_wr:
                        sync.wait_ge(s_wr[t], 16 * _nwrites(t))
                    elif do_scat:
                        sync.wait_ge(s_sc[t], 16 * _nwrites(t))
                sync.dma_start(out=dum_t[:, :], in_=v2t[0:1, 0:1]).then_inc(s_dum, 16)

        if two_queues and do_wr:

            @block.scalar
            def _(scalar):
                _writer(scalar, 1)

    nc.compile()
    nc.m = get_hw_module(nc.m)
    return nc


def prepare(edge_attr, emb_table, edge_index, batch_vec):
    """Host routing + program build (SBUF-compose path). Returns (nc, in_maps)."""
    buckets, cap, emb_np = _route(
        edge_attr, emb_table, edge_index, batch_vec, qsplit=4, sort=False
    )
    nc = _build_program_compose(cap)
    return nc, build_in_maps_compose(buckets, cap, emb_np)


def kernel(edge_attr, emb_table, edge_index, batch_vec):
    global LAST_EXEC_NS, LAST_RESULTS
    nc, in_maps = prepare(edge_attr, emb_table, edge_index, batch_vec)

    trace = bool(int(os.environ.get("BASSK_TRACE", "0")))
    res = run_bass_kernel_spmd(nc, in_maps, list(range(NCORES)), trace=trace)
    LAST_EXEC_NS = res.exec_time_ns
    LAST_RESULTS = res

    out = np.empty((B, N, N, D), np.float32)
    for core in range(NCORES):
        blockv = res.results[core]["out"].reshape(GPC, N, N, D)
        out[core * GPC : (core + 1) * GPC] = blockv
    return out
